# revision 10
# baseline (speedup 1.0000x reference)
"""3-layer GAT on 8 Trainium2 NeuronCores (Bass/Tile).

Strategy: partition nodes across the 8 cores (graph parallel); edges live with
their destination core so segment-softmax/aggregation stay local; per layer,
all-gather the (bf16) node features + attention source logits; gather source
rows per edge chunk with dma_gather; aggregate with one-hot selection matmuls
on the PE (single ldweights per 128-edge chunk, alpha applied to the gathered
rows on the DVE in fast mode, softmax denominator folded into the same rhs).

LayerNorm gammas are folded into the next layer's weights on the host
(leaky_relu is positive-homogeneous); zero biases skip the bias add; the
layer-3 softmax-denominator division is skipped entirely (LN scale
invariance).  The only activation functions used on the ACT engine are
{Exp, Ln, Copy}, which share one table set - no act-table reloads.

Self-contained: only imports the system concourse install.
"""

import os
import sys

for _p in ("/opt/trn_rl_repo", "/root/.axon_site/_ro/trn_rl_repo"):
    if os.path.isdir(_p) and _p not in sys.path:
        sys.path.insert(0, _p)

from dataclasses import dataclass

import ml_dtypes
import numpy as np

import concourse.bacc as bacc
import concourse.bass as bass
import concourse.tile as tile
from concourse import mybir
from concourse.bass_utils import run_bass_kernel_spmd

P = 128
BF16 = mybir.dt.bfloat16
F32 = mybir.dt.float32
I16 = mybir.dt.int16
AL = mybir.AluOpType
AF = mybir.ActivationFunctionType

NEG_SLOPE_ATT = 0.2
NEG_SLOPE_ACT = 0.01
LN_EPS = 1e-5


def _ceil(a, b):
    return -(-a // b)


def _ceil_arr(a, b):
    return -(-a // b)


def _pad_elem(n_f32_elems):
    """bf16 row length (elements) padded so row bytes are a multiple of 256."""
    return _ceil(n_f32_elems * 2, 256) * 128


@dataclass
class Cfg:
    N: int = 50000
    E: int = 400000
    F_IN: int = 256
    HEADS: int = 4
    C1: int = 256
    C2: int = 128
    NCLS: int = 32
    NCORES: int = 8

    def __post_init__(self):
        assert self.N % self.NCORES == 0
        self.NL = self.N // self.NCORES
        self.T = _ceil(self.NL, P)
        self.NLP = self.T * P
        self.NPTOT = self.NLP * self.NCORES
        assert self.NPTOT % 2 == 0
        self.HALF = self.NPTOT // 2
        assert self.HALF <= 32767, "half-table must be int16 addressable"
        H = self.HEADS
        self.CO1 = H * self.C1
        self.CO2 = H * self.C2
        assert self.F_IN % P == 0 and self.CO1 % P == 0 and self.CO2 % P == 0
        # bf16 hs-table rows: [h | s | pad], bytes % 256 == 0
        self.ELEM1 = _pad_elem(self.CO1 + H)
        self.ELEM2 = _pad_elem(self.CO2 + H)
        self.ELEM3 = _pad_elem(self.NCLS + 1)


@dataclass
class Meta:
    nch: list  # [T][2] chunk counts (common across cores)
    si: list   # [T][2] idx16 column offsets
    sc: list   # [T][2] dstloc column offsets
    SI: int
    SC: int
    sd: list = None   # [T] dst-idx column offsets (per-tile d gather)
    SD: int = 0
    bias12: bool = False  # layer-1/2 biases nonzero -> add brow in epilogue
    gfold: bool = True    # gammas folded into next-layer weights


def host_prep(cfg: Cfg, x, edge_src, edge_dst,
              W1, a_src1, a_dst1, b1, ln1_g, ln1_b,
              W2, a_src2, a_dst2, b2, ln2_g, ln2_b,
              W3, a_src3, a_dst3, b3, ln3_g, ln3_b):
    """Build per-core input maps + the (common) chunk structure."""
    c = cfg
    bf = ml_dtypes.bfloat16

    # ---- append self loops, shard edges by destination core
    loops = np.arange(c.N, dtype=np.int64)
    src = np.concatenate([edge_src.astype(np.int64), loops])
    dst = np.concatenate([edge_dst.astype(np.int64), loops])

    dst_core = dst // c.NL
    dstloc = dst - dst_core * c.NL
    tile_id = dstloc // P
    gsrc = (src // c.NL) * c.NLP + (src % c.NL)     # padded-global source row
    half = (gsrc >= c.HALF).astype(np.int64)
    idx16 = (gsrc - half * c.HALF).astype(np.int64)

    # group edges per (core, tile, half)
    counts = np.zeros((c.NCORES, c.T, 2), np.int64)
    np.add.at(counts, (dst_core, tile_id, half), 1)
    nch = np.maximum(_ceil_arr(counts.max(axis=0), P), 0)  # [T,2] chunks
    # offsets
    si = np.zeros((c.T, 2), np.int64)
    sc = np.zeros((c.T, 2), np.int64)
    acc_si = acc_sc = 0
    for t in range(c.T):
        for h in range(2):
            si[t, h] = acc_si
            sc[t, h] = acc_sc
            acc_si += int(nch[t, h]) * (P // 16)
            acc_sc += int(nch[t, h])
    SI, SC = int(acc_si), int(acc_sc)
    sd = np.zeros(c.T, np.int64)
    acc_sd = 0
    for t in range(c.T):
        sd[t] = acc_sd
        acc_sd += int(nch[t, 0] + nch[t, 1]) * (P // 16)
    SD = int(acc_sd)

    bias12 = bool(np.any(b1) or np.any(b2))
    gfold = bool(np.all(ln1_g > 0) and np.all(ln2_g > 0))
    meta = Meta(nch=nch.tolist(), si=si.tolist(), sc=sc.tolist(), SI=SI, SC=SC,
                sd=sd.tolist(), SD=SD, bias12=bias12, gfold=gfold)

    # ---- per-core index / dstloc tables
    order = np.lexsort((half, tile_id, dst_core))  # group by (core, tile, half)
    src_s, half_s, t_s, core_s = (idx16[order], half[order], tile_id[order],
                                  dst_core[order])
    dstrel_s = (dstloc - tile_id * P)[order]

    idx_tabs, dl_tabs = [], []
    # per-core group start offsets
    starts = np.zeros((c.NCORES, c.T, 2), np.int64)
    run = 0
    for cc in range(c.NCORES):
        for t in range(c.T):
            for h in range(2):
                starts[cc, t, h] = run
                run += int(counts[cc, t, h])
    dstidx_tabs = []
    for cc in range(c.NCORES):
        itab = np.zeros((16, SI), np.int16)
        dtab = np.full((P, SC), -1.0, np.float32)
        ditab = np.zeros((16, SD), np.int16)
        for t in range(c.T):
            dchunk = 0  # chunk index within the tile (across halves)
            for h in range(2):
                m = int(counts[cc, t, h])
                n = int(nch[t, h])
                if n == 0:
                    continue
                s0 = int(starts[cc, t, h])
                iv = np.zeros(n * P, np.int16)
                iv[:m] = src_s[s0:s0 + m].astype(np.int16)
                cols = int(si[t, h])
                blk = iv.reshape(n * P // 16, 16).T  # idx k -> [k%16, k//16]
                itab[:, cols:cols + n * (P // 16)] = blk
                dv = np.full(n * P, -1.0, np.float32)
                dv[:m] = dstrel_s[s0:s0 + m].astype(np.float32)
                dtab[:, sc[t, h]:sc[t, h] + n] = dv.reshape(n, P).T
                # dst-row indices (into the local [NLP] d table)
                div = np.zeros(n * P, np.int16)
                div[:m] = (t * P + dstrel_s[s0:s0 + m]).astype(np.int16)
                dc = int(sd[t]) + dchunk * (P // 16)
                ditab[:, dc:dc + n * (P // 16)] = div.reshape(
                    n * P // 16, 16).T
                dchunk += n
        idx_tabs.append(np.tile(itab, (8, 1)))
        dl_tabs.append(dtab)
        dstidx_tabs.append(np.tile(ditab, (8, 1)))

    # ---- weights (augmented with U = W.T @ a columns), bf16
    def aug(W, a_s, a_d, H, C, g_in=None):
        WT = W.T.astype(np.float64)                      # [Fin, H*C]
        U_s = np.zeros((WT.shape[0], H))
        U_d = np.zeros((WT.shape[0], H))
        for h in range(H):
            U_s[:, h] = WT[:, h * C:(h + 1) * C] @ a_s[h].astype(np.float64)
            U_d[:, h] = WT[:, h * C:(h + 1) * C] @ a_d[h].astype(np.float64)
        out = np.concatenate([WT, U_s, U_d], axis=1)
        if g_in is not None:
            out = out * g_in.astype(np.float64)[:, None]
        return out.astype(bf)

    g1 = ln1_g if gfold else None
    g2 = ln2_g if gfold else None
    W1a = aug(W1, a_src1, a_dst1, c.HEADS, c.C1)           # [F_IN, CO1+2H]
    W2a = aug(W2, a_src2, a_dst2, c.HEADS, c.C2, g1)       # [CO1, CO2+2H]
    W3a = aug(W3, a_src3, a_dst3, 1, c.NCLS, g2)           # [CO2, NCLS+2]

    def row(v):
        return np.repeat(np.asarray(v, np.float32)[None, :].astype(bf),
                         P, axis=0)

    brow1 = row(b1)
    brow2 = row(b2)
    grow1 = row(ln1_g)
    grow2 = row(ln2_g)
    g3b3 = np.concatenate([row(ln3_g), row(b3)], axis=1)   # [128, 2*NCLS]

    iota_b = np.repeat(np.arange(P, dtype=np.float32)[None, :],
                       P, axis=0).astype(bf)
    ident = np.eye(P, dtype=bf)

    in_maps = []
    for cc in range(c.NCORES):
        xl = np.zeros((c.NLP, c.F_IN), np.float32)
        xl[:c.NL] = x[cc * c.NL:(cc + 1) * c.NL]
        in_maps.append({
            "xT": np.ascontiguousarray(xl.T).astype(bf),
            "W1a": W1a, "W2a": W2a, "W3a": W3a,
            "brow1": brow1, "brow2": brow2,
            "grow1": grow1, "grow2": grow2, "g3b3": g3b3,
            "idx16": idx_tabs[cc], "dstloc": dl_tabs[cc],
            "dstidx16": dstidx_tabs[cc],
            "iotab": iota_b, "ident": ident,
        })
    return in_maps, meta


# --------------------------------------------------------------------------
# device program
# --------------------------------------------------------------------------

def build_nc(cfg: Cfg, meta: Meta, max_phase: int = 6):
    c = cfg
    H = c.HEADS
    nc = bacc.Bacc("TRN2", target_bir_lowering=False, debug=False,
                   num_devices=c.NCORES, enable_partition_id=False)

    eps_t = nc.alloc_sbuf_tensor("const-f32-lneps", [128, 1], F32)
    nc.gpsimd.memset(eps_t.ap(), LN_EPS)
    nc.const_aps.aps[(F32, LN_EPS)] = eps_t.ap()

    # ---- I/O
    xT = nc.dram_tensor("xT", [c.F_IN, c.NLP], BF16, kind="ExternalInput").ap()
    W1a = nc.dram_tensor("W1a", [c.F_IN, c.CO1 + 2 * H], BF16, kind="ExternalInput").ap()
    W2a = nc.dram_tensor("W2a", [c.CO1, c.CO2 + 2 * H], BF16, kind="ExternalInput").ap()
    W3a = nc.dram_tensor("W3a", [c.CO2, c.NCLS + 2], BF16, kind="ExternalInput").ap()
    brow1 = nc.dram_tensor("brow1", [P, c.CO1], BF16, kind="ExternalInput").ap()
    brow2 = nc.dram_tensor("brow2", [P, c.CO2], BF16, kind="ExternalInput").ap()
    grow1 = nc.dram_tensor("grow1", [P, c.CO1], BF16, kind="ExternalInput").ap()
    grow2 = nc.dram_tensor("grow2", [P, c.CO2], BF16, kind="ExternalInput").ap()
    g3b3 = nc.dram_tensor("g3b3", [P, 2 * c.NCLS], BF16, kind="ExternalInput").ap()
    idx16 = nc.dram_tensor("idx16", [P, meta.SI], I16, kind="ExternalInput").ap()
    dstloc = nc.dram_tensor("dstloc", [P, meta.SC], F32, kind="ExternalInput").ap()
    dstidx16 = nc.dram_tensor("dstidx16", [P, meta.SD], I16,
                              kind="ExternalInput").ap()
    iotab = nc.dram_tensor("iotab", [P, P], BF16, kind="ExternalInput").ap()
    ident = nc.dram_tensor("ident", [P, P], BF16, kind="ExternalInput").ap()
    y = nc.dram_tensor("y", [c.NLP, c.NCLS], F32, kind="ExternalOutput").ap()

    groups = [list(range(c.NCORES))]

    with tile.TileContext(nc) as tc:
        # ---- persistent DRAM intermediates
        dram_cm = tc.tile_pool(name="dram", bufs=1, space="DRAM")
        dram = dram_cm.__enter__()
        aspace = "Shared" if c.NCORES > 4 else "Local"
        hs1_loc = dram.tile([c.NLP, c.ELEM1], BF16)
        hs1_full = dram.tile([c.NPTOT, c.ELEM1], BF16, addr_space=aspace)
        hs2_loc = dram.tile([c.NLP, c.ELEM2], BF16)
        hs2_full = dram.tile([c.NPTOT, c.ELEM2], BF16, addr_space=aspace)
        hs3_loc = dram.tile([c.NLP, c.ELEM3], BF16)
        hs3_full = dram.tile([c.NPTOT, c.ELEM3], BF16, addr_space=aspace)
        # d tables with 256B rows for dma_gather (cols 0:H hold d, rest pad)
        d1t = dram.tile([c.NLP, P], BF16)
        d2t = dram.tile([c.NLP, P], BF16)
        d3t = dram.tile([c.NLP, P], BF16)

        # ---- persistent SBUF constants
        cpool_cm = tc.tile_pool(name="const", bufs=1)
        cpool = cpool_cm.__enter__()
        KC1 = c.F_IN // P
        xT_sb = cpool.tile([P, KC1 * c.NLP], BF16)
        for k in range(KC1):
            nc.sync.dma_start(xT_sb[:, k * c.NLP:(k + 1) * c.NLP],
                              xT[k * P:(k + 1) * P, :])
        W1w = c.CO1 + 2 * H
        W1a_sb = cpool.tile([P, KC1 * W1w], BF16)
        for k in range(KC1):
            nc.sync.dma_start(W1a_sb[:, k * W1w:(k + 1) * W1w],
                              W1a[k * P:(k + 1) * P, :])
        KC2 = c.CO1 // P
        W2w = c.CO2 + 2 * H
        W2a_sb = cpool.tile([P, KC2 * W2w], BF16)
        for k in range(KC2):
            nc.sync.dma_start(W2a_sb[:, k * W2w:(k + 1) * W2w],
                              W2a[k * P:(k + 1) * P, :])
        KC3 = c.CO2 // P
        W3w = c.NCLS + 2
        W3a_sb = cpool.tile([P, KC3 * W3w], BF16)
        for k in range(KC3):
            nc.sync.dma_start(W3a_sb[:, k * W3w:(k + 1) * W3w],
                              W3a[k * P:(k + 1) * P, :])
        brow1_sb = cpool.tile([P, c.CO1], BF16)
        nc.sync.dma_start(brow1_sb[:], brow1[:])
        brow2_sb = cpool.tile([P, c.CO2], BF16)
        nc.sync.dma_start(brow2_sb[:], brow2[:])
        grow1_sb = cpool.tile([P, c.CO1], BF16)
        nc.sync.dma_start(grow1_sb[:], grow1[:])
        grow2_sb = cpool.tile([P, c.CO2], BF16)
        nc.sync.dma_start(grow2_sb[:], grow2[:])
        g3b3_sb = cpool.tile([P, 2 * c.NCLS], BF16)
        nc.sync.dma_start(g3b3_sb[:], g3b3[:])
        idx_sb = cpool.tile([P, meta.SI], I16)
        nc.sync.dma_start(idx_sb[:], idx16[:])
        dl_sb = cpool.tile([P, meta.SC], F32)
        nc.sync.dma_start(dl_sb[:], dstloc[:])
        didx_sb = cpool.tile([P, meta.SD], I16)
        nc.sync.dma_start(didx_sb[:], dstidx16[:])
        iota_sb = cpool.tile([P, P], BF16)
        nc.sync.dma_start(iota_sb[:], iotab[:])
        id_sb = cpool.tile([P, P], BF16)
        nc.sync.dma_start(id_sb[:], ident[:])

        # ================= phase A: h1 = x @ W1a (per local node tile)
        with (
            tc.tile_pool(name="pA", bufs=3) as pA,
            tc.tile_pool(name="pAp", bufs=2, space="PSUM") as pAp,
        ):
            for t in range(c.T):
                hp = pAp.tile([P, W1w], F32, tag="hp")
                for k in range(KC1):
                    _mm_cols(nc, hp, xT_sb[:, k * c.NLP + t * P:
                                           k * c.NLP + (t + 1) * P],
                             W1a_sb[:, k * W1w:(k + 1) * W1w], W1w,
                             start=(k == 0), stop=(k == KC1 - 1))
                _store_hs(nc, pA, hp, c.CO1, H, c.ELEM1, hs1_loc, d1t, t)

        if c.NCORES == 1:
            hs1_full, hs2_full, hs3_full = hs1_loc, hs2_loc, hs3_loc
        if max_phase >= 1 and c.NCORES > 1:
            nc.gpsimd.collective_compute(
                "AllGather", AL.bypass, replica_groups=groups,
                ins=[hs1_loc[:].opt()], outs=[hs1_full[:].opt()])

        # ================= layer-1 aggregation + LN + fused L2 matmul
        if max_phase >= 2:
            _edge_phase(
                nc, tc, c, meta, lay=1, Hn=H, Ch=c.C1, ELEM=c.ELEM1,
                hs_full=hs1_full, d_tab=d1t,
                brow_sb=(brow1_sb if meta.bias12 else None),
                grow_sb=(None if meta.gfold else grow1_sb),
                g3b3_sb=None,
                iota_sb=iota_sb, id_sb=id_sb, idx_sb=idx_sb, dl_sb=dl_sb,
                didx_sb=didx_sb,
                fuse=dict(W_sb=W2a_sb, KC=KC2, Ww=W2w, CO=c.CO2, Hn2=H,
                          ELEMn=c.ELEM2, hs_loc=hs2_loc, d_next=d2t),
                final=None, y=None)

        if max_phase >= 3 and c.NCORES > 1:
            nc.gpsimd.collective_compute(
                "AllGather", AL.bypass, replica_groups=groups,
                ins=[hs2_loc[:].opt()], outs=[hs2_full[:].opt()])

        # ================= layer-2 aggregation + LN + fused L3 matmul
        if max_phase >= 4:
            _edge_phase(
                nc, tc, c, meta, lay=2, Hn=H, Ch=c.C2, ELEM=c.ELEM2,
                hs_full=hs2_full, d_tab=d2t,
                brow_sb=(brow2_sb if meta.bias12 else None),
                grow_sb=(None if meta.gfold else grow2_sb),
                g3b3_sb=None,
                iota_sb=iota_sb, id_sb=id_sb, idx_sb=idx_sb, dl_sb=dl_sb,
                didx_sb=didx_sb,
                fuse=dict(W_sb=W3a_sb, KC=KC3, Ww=W3w, CO=c.NCLS, Hn2=1,
                          ELEMn=c.ELEM3, hs_loc=hs3_loc, d_next=d3t),
                final=None, y=None)

        if max_phase >= 5 and c.NCORES > 1:
            nc.gpsimd.collective_compute(
                "AllGather", AL.bypass, replica_groups=groups,
                ins=[hs3_loc[:].opt()], outs=[hs3_full[:].opt()])

        # ================= layer-3 aggregation + LN + log_softmax
        if max_phase >= 6:
            _edge_phase(
                nc, tc, c, meta, lay=3, Hn=1, Ch=c.NCLS, ELEM=c.ELEM3,
                hs_full=hs3_full, d_tab=d3t,
                brow_sb=None, grow_sb=None, g3b3_sb=g3b3_sb,
                iota_sb=iota_sb, id_sb=id_sb, idx_sb=idx_sb, dl_sb=dl_sb,
                didx_sb=didx_sb,
                fuse=None, final=True, y=y)

        cpool_cm.__exit__(None, None, None)
        dram_cm.__exit__(None, None, None)

    nc.compile()
    return nc


def _splits(W):
    out = []
    n0 = 0
    while n0 < W:
        nsz = min(512, W - n0)
        out.append((n0, nsz))
        n0 += nsz
    return out


def _mm_cols(nc, out_ps, lhsT, rhs, Ww, start, stop):
    """out_ps[:, :Ww] (+)= lhsT.T @ rhs, N split at 512."""
    for (n0, nsz) in _splits(Ww):
        nc.tensor.matmul(out=out_ps[:, n0:n0 + nsz], lhsT=lhsT,
                         rhs=rhs[:, n0:n0 + nsz], start=start, stop=stop)


def _mm_splits(nc, out_ps, lhs_sb, w_sb, KC, Ww, Plhs):
    """out_ps[:, :Ww] = sum_k lhs_k.T @ W_k, with N split at 512."""
    for k in range(KC):
        for (n0, nsz) in _splits(Ww):
            nc.tensor.matmul(
                out=out_ps[:, n0:n0 + nsz],
                lhsT=lhs_sb[:, k * Plhs:(k + 1) * Plhs],
                rhs=w_sb[:, k * Ww + n0:k * Ww + n0 + nsz],
                start=(k == 0), stop=(k == KC - 1))


def _store_hs(nc, pool, hp, CO, Hn, ELEM, hs_loc, d_tab, t):
    """PSUM [128, CO+2H] -> bf16 hs row tile + bf16 d table row tile."""
    hst = pool.tile([P, ELEM], BF16, tag="hst")
    nc.scalar.copy(hst[:, 0:CO], hp[:, 0:CO])
    nc.vector.tensor_copy(hst[:, CO:CO + Hn], hp[:, CO:CO + Hn])
    if ELEM > CO + Hn:
        nc.vector.memset(hst[:, CO + Hn:ELEM], 0)
    dt = pool.tile([P, P], BF16, tag="dt")
    nc.vector.tensor_copy(dt[:, 0:Hn], hp[:, CO + Hn:CO + 2 * Hn])
    nc.vector.memset(dt[:, Hn:P], 0)
    nc.sync.dma_start(hs_loc[t * P:(t + 1) * P, :], hst[:])
    nc.sync.dma_start(d_tab[t * P:(t + 1) * P, :], dt[:])


def _edge_phase(nc, tc, c: Cfg, meta: Meta, lay, Hn, Ch, ELEM, hs_full, d_tab,
                brow_sb, grow_sb, g3b3_sb, iota_sb, id_sb, idx_sb, dl_sb,
                didx_sb, fuse, final, y):
    CO = Hn * Ch
    max_nch = max(max(r) for r in meta.nch)
    max_ntot = max(r[0] + r[1] for r in meta.nch)
    use_den = not final           # L3: LN is invariant to the row scale
    AW = CO + (Hn if use_den else 0)   # Gw width (den cols at the tail)

    with (
        tc.tile_pool(name=f"sb{lay}", bufs=2) as sb,
        tc.tile_pool(name=f"sc{lay}", bufs=4) as sbc,
        tc.tile_pool(name=f"g{lay}", bufs=4) as gp,
        tc.tile_pool(name=f"gw{lay}", bufs=4) as gw,
        tc.tile_pool(name=f"ps{lay}", bufs=1, space="PSUM") as ps1,
        tc.tile_pool(name=f"psagg{lay}", bufs=2, space="PSUM") as psA,
    ):
        for t in range(c.T):
            nch0, nch1 = meta.nch[t]
            ntot = nch0 + nch1
            agg = psA.tile([P, CO], F32, tag="agg")
            den = (ps1.tile([P, Hn], F32, tag="den", name=f"den{lay}")
                   if use_den else None)
            if ntot == 0:
                nc.vector.memset(agg[:], 0)
                if use_den:
                    nc.vector.memset(den[:], 0)
                _epilogue(nc, sb, ps1, c, meta, lay, t, agg, den, Hn, Ch, CO,
                          use_den, brow_sb, grow_sb, g3b3_sb, id_sb,
                          fuse, final, y)
                continue

            # per-edge d rows for the tile (256B rows from the d table);
            # split per half to stay under the 64-descriptor packet limit
            Dg = gp.tile([P, max_ntot * P], BF16, tag="Dg")
            for hf, nch in ((0, nch0), (1, nch1)):
                if nch == 0:
                    continue
                b0 = 0 if hf == 0 else nch0
                sdo = meta.sd[t] + b0 * (P // 16)
                nc.gpsimd.dma_gather(
                    out_ap=Dg[:, b0 * P:(b0 + nch) * P].rearrange(
                        "p (k d) -> p k d", d=P),
                    in_ap=d_tab[:],
                    idxs_ap=didx_sb[:, sdo:sdo + nch * (P // 16)],
                    num_idxs=nch * P, num_idxs_reg=nch * P, elem_size=P)

            Gs = []
            for hf, nch in ((0, nch0), (1, nch1)):
                if nch == 0:
                    Gs.append(None)
                    continue
                G = gp.tile([P, max_nch * ELEM], BF16, tag="G")
                si = meta.si[t][hf]
                nidx = nch * P
                nc.gpsimd.dma_gather(
                    out_ap=G[:, 0:nch * ELEM].rearrange(
                        "p (k d) -> p k d", d=ELEM),
                    in_ap=hs_full[hf * c.HALF:(hf + 1) * c.HALF, :],
                    idxs_ap=idx_sb[:, si:si + nch * (P // 16)],
                    num_idxs=nidx, num_idxs_reg=nidx, elem_size=ELEM)
                Gs.append(G)

            # ---- per-tile prep: tsd = s_src + d_dst, leaky, exp
            tsda = sbc.tile([P, max_ntot * Hn], F32, tag="tsda")
            for hf, nch in ((0, nch0), (1, nch1)):
                if nch == 0:
                    continue
                b0 = 0 if hf == 0 else nch0
                Gv = Gs[hf][:, 0:nch * ELEM].rearrange(
                    "p (k d) -> p k d", d=ELEM)[:, :, CO:CO + Hn]
                Dv = Dg[:, b0 * P:(b0 + nch) * P].rearrange(
                    "p (k d) -> p k d", d=P)[:, :, 0:Hn]
                nc.vector.tensor_tensor(
                    out=tsda[:, b0 * Hn:(b0 + nch) * Hn].rearrange(
                        "p (k h) -> p k h", h=Hn),
                    in0=Gv, in1=Dv, op=AL.add)
            lra = sbc.tile([P, max_ntot * Hn], F32, tag="lra")
            nc.vector.scalar_tensor_tensor(
                out=lra[:, 0:ntot * Hn], in0=tsda[:, 0:ntot * Hn],
                scalar=NEG_SLOPE_ATT, in1=tsda[:, 0:ntot * Hn],
                op0=AL.mult, op1=AL.max)
            wfa = sbc.tile([P, max_ntot * Hn], F32, tag="wfa")
            nc.scalar.activation(wfa[:, 0:ntot * Hn], lra[:, 0:ntot * Hn],
                                 AF.Exp)
            if use_den:
                wfb = sbc.tile([P, max_ntot * Hn], BF16, tag="wfb")
                nc.vector.tensor_copy(wfb[:, 0:ntot * Hn],
                                      wfa[:, 0:ntot * Hn])

            # ---- per chunk: one-hot lhsT, alpha-scaled rhs, matmuls
            c0 = meta.sc[t][0]
            sp = _splits(CO)
            gchunk = 0
            for hf, nch in ((0, nch0), (1, nch1)):
                G = Gs[hf]
                for b in range(nch):
                    first = (gchunk == 0)
                    last = (gchunk == ntot - 1)
                    eq = gw.tile([P, P], BF16, tag="eq")
                    nc.vector.tensor_scalar(
                        out=eq[:], in0=iota_sb[:],
                        scalar1=dl_sb[:, c0 + gchunk:c0 + gchunk + 1],
                        scalar2=None, op0=AL.is_equal)
                    Gw = gw.tile([P, AW], BF16, tag="Gw")
                    for h in range(Hn):
                        nc.vector.tensor_scalar(
                            out=Gw[:, h * Ch:(h + 1) * Ch],
                            in0=G[:, b * ELEM + h * Ch:b * ELEM + (h + 1) * Ch],
                            scalar1=wfa[:, gchunk * Hn + h:gchunk * Hn + h + 1],
                            scalar2=None, op0=AL.mult)
                    if use_den:
                        nc.vector.tensor_copy(
                            Gw[:, CO:CO + Hn],
                            wfb[:, gchunk * Hn:(gchunk + 1) * Hn])
                    for (n0, nsz) in sp:
                        nc.tensor.matmul(out=agg[:, n0:n0 + nsz], lhsT=eq[:],
                                         rhs=Gw[:, n0:n0 + nsz],
                                         start=first, stop=last)
                    if use_den:
                        nc.tensor.matmul(out=den[:], lhsT=eq[:],
                                         rhs=Gw[:, CO:CO + Hn],
                                         start=first, stop=last)
                    gchunk += 1

            _epilogue(nc, sb, ps1, c, meta, lay, t, agg, den, Hn, Ch, CO,
                      use_den, brow_sb, grow_sb, g3b3_sb, id_sb,
                      fuse, final, y)


def _epilogue(nc, sb, ps1, c, meta, lay, t, agg, den, Hn, Ch, CO, use_den,
              brow_sb, grow_sb, g3b3_sb, id_sb, fuse, final, y):
    # normalize by the softmax denominator (if needed), then LayerNorm with
    # gamma folded into the next weights; leaky + fused next-layer matmul,
    # or log_softmax for the final layer.
    ob = sb.tile([P, CO], BF16, tag="ob")
    rs = sb.tile([P, 1], F32, tag="rs")
    if use_den:
        denr = sb.tile([P, Hn], F32, tag="denr")
        nc.vector.tensor_scalar(out=denr[:], in0=den[:],
                                scalar1=1e-16, scalar2=None, op0=AL.add)
        rec = sb.tile([P, Hn], F32, tag="rec")
        nc.vector.reciprocal(rec[:], denr[:])
        nc.vector.tensor_tensor(
            out=ob[:].rearrange("p (h c) -> p h c", h=Hn),
            in0=agg[:].rearrange("p (h c) -> p h c", h=Hn),
            in1=rec[:].to_broadcast([P, Hn, Ch]), op=AL.mult)
        nc.vector.tensor_reduce(out=rs[:], in_=ob[:],
                                axis=mybir.AxisListType.X, op=AL.add)
    else:
        nc.vector.tensor_scalar(out=ob[:], in0=agg[:, 0:CO], scalar1=1.0,
                                scalar2=0.0, op0=AL.mult, op1=AL.add,
                                accum_out=rs[:])
    if brow_sb is not None:
        ob2 = sb.tile([P, CO], BF16, tag="ob2")
        nc.vector.tensor_tensor(out=ob2[:], in0=ob[:], in1=brow_sb[:],
                                op=AL.add)
        ob = ob2
        rs2 = sb.tile([P, 1], F32, tag="rs2")
        nc.vector.tensor_reduce(out=rs2[:], in_=ob[:],
                                axis=mybir.AxisListType.X, op=AL.add)
        rs = rs2
    # LayerNorm statistics: mean, then var = sum((x-mu)*x)/CO
    nm = sb.tile([P, 1], F32, tag="nm")
    nc.vector.tensor_scalar(out=nm[:], in0=rs[:], scalar1=1.0 / CO,
                            scalar2=None, op0=AL.mult)
    sqs = sb.tile([P, CO], BF16, tag="sqs")
    vs = sb.tile([P, 1], F32, tag="vs")
    nc.vector.scalar_tensor_tensor(
        out=sqs[:], in0=ob[:], scalar=nm[:, 0:1], in1=ob[:],
        op0=AL.subtract, op1=AL.mult, accum_out=vs[:])
    lnv = sb.tile([P, 1], F32, tag="lnv")
    nc.scalar.activation(lnv[:], vs[:], AF.Ln, bias=LN_EPS, scale=1.0 / CO)
    rstd = sb.tile([P, 1], F32, tag="rstd")
    nc.scalar.activation(rstd[:], lnv[:], AF.Exp, bias=0.0, scale=-0.5)
    y1 = sb.tile([P, CO], BF16, tag="y1")
    nc.vector.tensor_scalar(out=y1[:], in0=ob[:], scalar1=nm[:, 0:1],
                            scalar2=rstd[:, 0:1], op0=AL.subtract,
                            op1=AL.mult)
    if grow_sb is not None:
        yg = sb.tile([P, CO], BF16, tag="yg")
        nc.vector.tensor_tensor(out=yg[:], in0=y1[:], in1=grow_sb[:],
                                op=AL.mult)
        y1 = yg

    if final:
        # y2 = y1*g3 + b3, then log_softmax over CO, write y (f32 math)
        yg3 = sb.tile([P, CO], F32, tag="yg3")
        nc.vector.tensor_tensor(out=yg3[:], in0=y1[:],
                                in1=g3b3_sb[:, 0:CO], op=AL.mult)
        yb3 = sb.tile([P, CO], F32, tag="yb3")
        nc.vector.tensor_tensor(out=yb3[:], in0=yg3[:],
                                in1=g3b3_sb[:, CO:2 * CO], op=AL.add)
        mx = sb.tile([P, 1], F32, tag="mx")
        nc.vector.tensor_reduce(out=mx[:], in_=yb3[:],
                                axis=mybir.AxisListType.X, op=AL.max)
        xs = sb.tile([P, CO], F32, tag="xs")
        nc.vector.tensor_scalar(out=xs[:], in0=yb3[:], scalar1=mx[:, 0:1],
                                scalar2=None, op0=AL.subtract)
        ex = sb.tile([P, CO], F32, tag="ex")
        se = sb.tile([P, 1], F32, tag="se")
        nc.scalar.activation(ex[:], xs[:], AF.Exp, accum_out=se[:])
        lse = sb.tile([P, 1], F32, tag="lse")
        nc.scalar.activation(lse[:], se[:], AF.Ln)
        yo = sb.tile([P, CO], F32, tag="yo")
        nc.vector.tensor_scalar(out=yo[:], in0=xs[:], scalar1=lse[:, 0:1],
                                scalar2=None, op0=AL.subtract)
        nc.sync.dma_start(y[t * P:(t + 1) * P, :], yo[:])
        return

    # leaky(0.01) -> bf16 x_next; fused next-layer matmul
    x2 = sb.tile([P, CO], BF16, tag="x2")
    nc.vector.scalar_tensor_tensor(
        out=x2[:], in0=y1[:], scalar=NEG_SLOPE_ACT, in1=y1[:],
        op0=AL.mult, op1=AL.max)
    W_sb, KC, Ww = fuse["W_sb"], fuse["KC"], fuse["Ww"]
    CO2, Hn2, ELEMn = fuse["CO"], fuse["Hn2"], fuse["ELEMn"]
    xt2 = sb.tile([P, KC * P], BF16, tag="xt2")
    for k in range(KC):
        scr = ps1.tile([P, P], BF16, tag="scr")
        nc.tensor.transpose(out=scr[:], in_=x2[:, k * P:(k + 1) * P],
                            identity=id_sb[:])
        nc.scalar.copy(xt2[:, k * P:(k + 1) * P], scr[:])
    hp = ps1.tile([P, Ww], F32, tag="hnext")
    _mm_splits(nc, hp, xt2, W_sb, KC, Ww, P)
    _store_hs(nc, sb, hp, CO2, Hn2, ELEMn, fuse["hs_loc"], fuse["d_next"], t)


# --------------------------------------------------------------------------
# entry point
# --------------------------------------------------------------------------

_CACHE = {}


def _get_nc(cfg, meta):
    key = (tuple(sorted(cfg.__dict__.items())),
           tuple(tuple(r) for r in meta.nch), meta.bias12, meta.gfold)
    if key not in _CACHE:
        _CACHE[key] = build_nc(cfg, meta)
    return _CACHE[key]


def kernel(**inputs):
    inputs = {k: np.asarray(v) for k, v in inputs.items()}
    x = inputs["x"]
    cfg = Cfg(N=x.shape[0], E=inputs["edge_src"].shape[0], F_IN=x.shape[1],
              HEADS=inputs["a_src1"].shape[0], C1=inputs["a_src1"].shape[1],
              C2=inputs["a_src2"].shape[1], NCLS=inputs["W3"].shape[0],
              NCORES=8)
    in_maps, meta = host_prep(cfg, **inputs)
    nc = _get_nc(cfg, meta)
    trace = bool(int(os.environ.get("GAT_TRACE", "0")))
    res = run_bass_kernel_spmd(nc, in_maps, core_ids=list(range(cfg.NCORES)),
                               trace=trace)
    global LAST_EXEC_NS
    LAST_EXEC_NS = res.exec_time_ns
    out = np.concatenate(
        [res.results[cc]["y"][:cfg.NL] for cc in range(cfg.NCORES)], axis=0)
    return out.astype(np.float32)


LAST_EXEC_NS = None


if __name__ == "__main__":
    pass


# revision 12
# speedup vs baseline: 1.2604x; 1.2604x over previous
"""3-layer GAT on 8 Trainium2 NeuronCores (Bass/Tile).

Strategy: partition nodes across the 8 cores (graph parallel); edges live with
their destination core so segment-softmax/aggregation stay local; per layer,
all-gather the (bf16) node features + attention source logits; gather source
rows per edge chunk with dma_gather; aggregate with one-hot selection matmuls
on the PE (single ldweights per 128-edge chunk, alpha applied to the gathered
rows on the DVE in fast mode, softmax denominator folded into the same rhs).

LayerNorm gammas are folded into the next layer's weights on the host
(leaky_relu is positive-homogeneous); zero biases skip the bias add; the
layer-3 softmax-denominator division is skipped entirely (LN scale
invariance).  The only activation functions used on the ACT engine are
{Exp, Ln, Copy}, which share one table set - no act-table reloads.

Self-contained: only imports the system concourse install.
"""

import os
import sys

for _p in ("/opt/trn_rl_repo", "/root/.axon_site/_ro/trn_rl_repo"):
    if os.path.isdir(_p) and _p not in sys.path:
        sys.path.insert(0, _p)

from dataclasses import dataclass

import ml_dtypes
import numpy as np

import concourse.bacc as bacc
import concourse.bass as bass
import concourse.tile as tile
from concourse import mybir
from concourse.bass_utils import run_bass_kernel_spmd

_orig_get_act_tables = bacc.get_activation_tables


def _patched_act_tables(arch):
    tabs = dict(_orig_get_act_tables(arch))
    AFT = mybir.ActivationFunctionType
    combined = None
    for name, fns in tabs.items():
        if AFT.Exp in fns and AFT.Ln in fns:
            combined = name
            break
    if combined is not None:
        for name in list(tabs):
            if name != combined:
                tabs[name] = tabs[name] - {AFT.Exp, AFT.Ln}
    return tabs


bacc.get_activation_tables = _patched_act_tables

P = 128
BF16 = mybir.dt.bfloat16
F32 = mybir.dt.float32
I16 = mybir.dt.int16
AL = mybir.AluOpType
AF = mybir.ActivationFunctionType

NEG_SLOPE_ATT = 0.2
NEG_SLOPE_ACT = 0.01
LN_EPS = 1e-5


def _ceil(a, b):
    return -(-a // b)


def _ceil_arr(a, b):
    return -(-a // b)


def _pad_elem(n_f32_elems):
    """bf16 row length (elements) padded so row bytes are a multiple of 256."""
    return _ceil(n_f32_elems * 2, 256) * 128


@dataclass
class Cfg:
    N: int = 50000
    E: int = 400000
    F_IN: int = 256
    HEADS: int = 4
    C1: int = 256
    C2: int = 128
    NCLS: int = 32
    NCORES: int = 8

    def __post_init__(self):
        assert self.N % self.NCORES == 0
        self.NL = self.N // self.NCORES
        self.T = _ceil(self.NL, P)
        self.NLP = self.T * P
        self.NPTOT = self.NLP * self.NCORES
        assert self.NPTOT % 2 == 0
        self.HALF = self.NPTOT // 2
        assert self.HALF <= 32767, "half-table must be int16 addressable"
        H = self.HEADS
        self.CO1 = H * self.C1
        self.CO2 = H * self.C2
        assert self.F_IN % P == 0 and self.CO1 % P == 0 and self.CO2 % P == 0
        # bf16 hs-table rows: [h | s | pad], bytes % 256 == 0
        self.ELEM1 = _pad_elem(self.CO1 + H)
        self.ELEM2 = _pad_elem(self.CO2 + H)
        self.ELEM3 = _pad_elem(self.NCLS + 1)


@dataclass
class Meta:
    nch: list  # [T][2] chunk counts (common across cores)
    si: list   # [T][2] idx16 column offsets
    sc: list   # [T][2] dstloc column offsets
    SI: int
    SC: int
    sd: list = None   # [T] dst-idx column offsets (per-tile d gather)
    SD: int = 0
    bias12: bool = False  # layer-1/2 biases nonzero -> add brow in epilogue
    gfold: bool = True    # gammas folded into next-layer weights


def host_prep(cfg: Cfg, x, edge_src, edge_dst,
              W1, a_src1, a_dst1, b1, ln1_g, ln1_b,
              W2, a_src2, a_dst2, b2, ln2_g, ln2_b,
              W3, a_src3, a_dst3, b3, ln3_g, ln3_b):
    """Build per-core input maps + the (common) chunk structure."""
    c = cfg
    bf = ml_dtypes.bfloat16

    # ---- append self loops, shard edges by destination core
    loops = np.arange(c.N, dtype=np.int64)
    src = np.concatenate([edge_src.astype(np.int64), loops])
    dst = np.concatenate([edge_dst.astype(np.int64), loops])

    dst_core = dst // c.NL
    dstloc = dst - dst_core * c.NL
    tile_id = dstloc // P
    gsrc = (src // c.NL) * c.NLP + (src % c.NL)     # padded-global source row
    half = (gsrc >= c.HALF).astype(np.int64)
    idx16 = (gsrc - half * c.HALF).astype(np.int64)

    # group edges per (core, tile, half)
    counts = np.zeros((c.NCORES, c.T, 2), np.int64)
    np.add.at(counts, (dst_core, tile_id, half), 1)
    nch = np.maximum(_ceil_arr(counts.max(axis=0), P), 0)  # [T,2] chunks
    # offsets
    si = np.zeros((c.T, 2), np.int64)
    sc = np.zeros((c.T, 2), np.int64)
    acc_si = acc_sc = 0
    for t in range(c.T):
        for h in range(2):
            si[t, h] = acc_si
            sc[t, h] = acc_sc
            acc_si += int(nch[t, h]) * (P // 16)
            acc_sc += int(nch[t, h])
    SI, SC = int(acc_si), int(acc_sc)
    sd = np.zeros(c.T, np.int64)
    acc_sd = 0
    for t in range(c.T):
        sd[t] = acc_sd
        acc_sd += int(nch[t, 0] + nch[t, 1]) * (P // 16)
    SD = int(acc_sd)

    bias12 = bool(np.any(b1) or np.any(b2))
    gfold = bool(np.all(ln1_g > 0) and np.all(ln2_g > 0))
    meta = Meta(nch=nch.tolist(), si=si.tolist(), sc=sc.tolist(), SI=SI, SC=SC,
                sd=sd.tolist(), SD=SD, bias12=bias12, gfold=gfold)

    # ---- per-core index / dstloc tables
    order = np.lexsort((half, tile_id, dst_core))  # group by (core, tile, half)
    src_s, half_s, t_s, core_s = (idx16[order], half[order], tile_id[order],
                                  dst_core[order])
    dstrel_s = (dstloc - tile_id * P)[order]

    idx_tabs, dl_tabs = [], []
    # per-core group start offsets
    starts = np.zeros((c.NCORES, c.T, 2), np.int64)
    run = 0
    for cc in range(c.NCORES):
        for t in range(c.T):
            for h in range(2):
                starts[cc, t, h] = run
                run += int(counts[cc, t, h])
    dstidx_tabs = []
    for cc in range(c.NCORES):
        itab = np.zeros((16, SI), np.int16)
        dtab = np.full((P, SC), -1.0, np.float32)
        ditab = np.zeros((16, SD), np.int16)
        for t in range(c.T):
            dchunk = 0  # chunk index within the tile (across halves)
            for h in range(2):
                m = int(counts[cc, t, h])
                n = int(nch[t, h])
                if n == 0:
                    continue
                s0 = int(starts[cc, t, h])
                iv = np.zeros(n * P, np.int16)
                iv[:m] = src_s[s0:s0 + m].astype(np.int16)
                cols = int(si[t, h])
                blk = iv.reshape(n * P // 16, 16).T  # idx k -> [k%16, k//16]
                itab[:, cols:cols + n * (P // 16)] = blk
                dv = np.full(n * P, -1.0, np.float32)
                dv[:m] = dstrel_s[s0:s0 + m].astype(np.float32)
                dtab[:, sc[t, h]:sc[t, h] + n] = dv.reshape(n, P).T
                # dst-row indices (into the local [NLP] d table)
                div = np.zeros(n * P, np.int16)
                div[:m] = (t * P + dstrel_s[s0:s0 + m]).astype(np.int16)
                dc = int(sd[t]) + dchunk * (P // 16)
                ditab[:, dc:dc + n * (P // 16)] = div.reshape(
                    n * P // 16, 16).T
                dchunk += n
        idx_tabs.append(np.tile(itab, (8, 1)))
        dl_tabs.append(dtab)
        dstidx_tabs.append(np.tile(ditab, (8, 1)))

    # ---- weights (augmented with U = W.T @ a columns), bf16
    def aug(W, a_s, a_d, H, C, g_in=None):
        WT = W.T.astype(np.float64)                      # [Fin, H*C]
        U_s = np.zeros((WT.shape[0], H))
        U_d = np.zeros((WT.shape[0], H))
        for h in range(H):
            U_s[:, h] = WT[:, h * C:(h + 1) * C] @ a_s[h].astype(np.float64)
            U_d[:, h] = WT[:, h * C:(h + 1) * C] @ a_d[h].astype(np.float64)
        out = np.concatenate([WT, U_s, U_d], axis=1)
        if g_in is not None:
            out = out * g_in.astype(np.float64)[:, None]
        return out.astype(bf)

    g1 = ln1_g if gfold else None
    g2 = ln2_g if gfold else None
    W1a = aug(W1, a_src1, a_dst1, c.HEADS, c.C1)           # [F_IN, CO1+2H]
    W2a = aug(W2, a_src2, a_dst2, c.HEADS, c.C2, g1)       # [CO1, CO2+2H]
    W3a = aug(W3, a_src3, a_dst3, 1, c.NCLS, g2)           # [CO2, NCLS+2]

    def row(v):
        return np.repeat(np.asarray(v, np.float32)[None, :].astype(bf),
                         P, axis=0)

    brow1 = row(b1)
    brow2 = row(b2)
    grow1 = row(ln1_g)
    grow2 = row(ln2_g)
    g3b3 = np.concatenate([row(ln3_g), row(b3)], axis=1)   # [128, 2*NCLS]

    iota_b = np.repeat(np.arange(P, dtype=np.float32)[None, :],
                       P, axis=0).astype(bf)
    ident = np.eye(P, dtype=bf)

    in_maps = []
    for cc in range(c.NCORES):
        xl = np.zeros((c.NLP, c.F_IN), np.float32)
        xl[:c.NL] = x[cc * c.NL:(cc + 1) * c.NL]
        in_maps.append({
            "xT": np.ascontiguousarray(xl.T).astype(bf),
            "W1a": W1a, "W2a": W2a, "W3a": W3a,
            "brow1": brow1, "brow2": brow2,
            "grow1": grow1, "grow2": grow2, "g3b3": g3b3,
            "idx16": idx_tabs[cc], "dstloc": dl_tabs[cc],
            "dstidx16": dstidx_tabs[cc],
            "iotab": iota_b, "ident": ident,
        })
    return in_maps, meta


# --------------------------------------------------------------------------
# device program
# --------------------------------------------------------------------------

def build_nc(cfg: Cfg, meta: Meta, max_phase: int = 6):
    c = cfg
    H = c.HEADS
    nc = bacc.Bacc("TRN2", target_bir_lowering=False, debug=False,
                   num_devices=c.NCORES, enable_partition_id=False)

    eps_t = nc.alloc_sbuf_tensor("const-f32-lneps", [128, 1], F32)
    nc.gpsimd.memset(eps_t.ap(), LN_EPS)
    nc.const_aps.aps[(F32, LN_EPS)] = eps_t.ap()

    # ---- I/O
    xT = nc.dram_tensor("xT", [c.F_IN, c.NLP], BF16, kind="ExternalInput").ap()
    W1a = nc.dram_tensor("W1a", [c.F_IN, c.CO1 + 2 * H], BF16, kind="ExternalInput").ap()
    W2a = nc.dram_tensor("W2a", [c.CO1, c.CO2 + 2 * H], BF16, kind="ExternalInput").ap()
    W3a = nc.dram_tensor("W3a", [c.CO2, c.NCLS + 2], BF16, kind="ExternalInput").ap()
    brow1 = nc.dram_tensor("brow1", [P, c.CO1], BF16, kind="ExternalInput").ap()
    brow2 = nc.dram_tensor("brow2", [P, c.CO2], BF16, kind="ExternalInput").ap()
    grow1 = nc.dram_tensor("grow1", [P, c.CO1], BF16, kind="ExternalInput").ap()
    grow2 = nc.dram_tensor("grow2", [P, c.CO2], BF16, kind="ExternalInput").ap()
    g3b3 = nc.dram_tensor("g3b3", [P, 2 * c.NCLS], BF16, kind="ExternalInput").ap()
    idx16 = nc.dram_tensor("idx16", [P, meta.SI], I16, kind="ExternalInput").ap()
    dstloc = nc.dram_tensor("dstloc", [P, meta.SC], F32, kind="ExternalInput").ap()
    dstidx16 = nc.dram_tensor("dstidx16", [P, meta.SD], I16,
                              kind="ExternalInput").ap()
    iotab = nc.dram_tensor("iotab", [P, P], BF16, kind="ExternalInput").ap()
    ident = nc.dram_tensor("ident", [P, P], BF16, kind="ExternalInput").ap()
    y = nc.dram_tensor("y", [c.NLP, c.NCLS], F32, kind="ExternalOutput").ap()

    groups = [list(range(c.NCORES))]

    with tile.TileContext(nc) as tc:
        # ---- persistent DRAM intermediates
        dram_cm = tc.tile_pool(name="dram", bufs=1, space="DRAM")
        dram = dram_cm.__enter__()
        aspace = "Shared" if c.NCORES > 4 else "Local"
        hs1_loc = dram.tile([c.NLP, c.ELEM1], BF16)
        hs1_full = dram.tile([c.NPTOT, c.ELEM1], BF16, addr_space=aspace)
        hs2_loc = dram.tile([c.NLP, c.ELEM2], BF16)
        hs2_full = dram.tile([c.NPTOT, c.ELEM2], BF16, addr_space=aspace)
        hs3_loc = dram.tile([c.NLP, c.ELEM3], BF16)
        hs3_full = dram.tile([c.NPTOT, c.ELEM3], BF16, addr_space=aspace)
        # d tables with 256B rows for dma_gather (cols 0:H hold d, rest pad)
        d1t = dram.tile([c.NLP, P], BF16)
        d2t = dram.tile([c.NLP, P], BF16)
        d3t = dram.tile([c.NLP, P], BF16)

        # ---- persistent SBUF constants
        cpool_cm = tc.tile_pool(name="const", bufs=1)
        cpool = cpool_cm.__enter__()
        KC1 = c.F_IN // P
        xT_sb = cpool.tile([P, KC1 * c.NLP], BF16)
        for k in range(KC1):
            nc.sync.dma_start(xT_sb[:, k * c.NLP:(k + 1) * c.NLP],
                              xT[k * P:(k + 1) * P, :])
        W1w = c.CO1 + 2 * H
        W1a_sb = cpool.tile([P, KC1 * W1w], BF16)
        for k in range(KC1):
            nc.sync.dma_start(W1a_sb[:, k * W1w:(k + 1) * W1w],
                              W1a[k * P:(k + 1) * P, :])
        KC2 = c.CO1 // P
        W2w = c.CO2 + 2 * H
        W2a_sb = cpool.tile([P, KC2 * W2w], BF16)
        for k in range(KC2):
            nc.sync.dma_start(W2a_sb[:, k * W2w:(k + 1) * W2w],
                              W2a[k * P:(k + 1) * P, :])
        KC3 = c.CO2 // P
        W3w = c.NCLS + 2
        W3a_sb = cpool.tile([P, KC3 * W3w], BF16)
        for k in range(KC3):
            nc.sync.dma_start(W3a_sb[:, k * W3w:(k + 1) * W3w],
                              W3a[k * P:(k + 1) * P, :])
        brow1_sb = cpool.tile([P, c.CO1], BF16)
        nc.sync.dma_start(brow1_sb[:], brow1[:])
        brow2_sb = cpool.tile([P, c.CO2], BF16)
        nc.sync.dma_start(brow2_sb[:], brow2[:])
        grow1_sb = cpool.tile([P, c.CO1], BF16)
        nc.sync.dma_start(grow1_sb[:], grow1[:])
        grow2_sb = cpool.tile([P, c.CO2], BF16)
        nc.sync.dma_start(grow2_sb[:], grow2[:])
        g3b3_sb = cpool.tile([P, 2 * c.NCLS], BF16)
        nc.sync.dma_start(g3b3_sb[:], g3b3[:])
        idx_sb = cpool.tile([P, meta.SI], I16)
        nc.sync.dma_start(idx_sb[:], idx16[:])
        dl_sb = cpool.tile([P, meta.SC], F32)
        nc.sync.dma_start(dl_sb[:], dstloc[:])
        didx_sb = cpool.tile([P, meta.SD], I16)
        nc.sync.dma_start(didx_sb[:], dstidx16[:])
        iota_sb = cpool.tile([P, P], BF16)
        nc.sync.dma_start(iota_sb[:], iotab[:])
        id_sb = cpool.tile([P, P], BF16)
        nc.sync.dma_start(id_sb[:], ident[:])

        # ================= phase A: h1 = x @ W1a (per local node tile)
        with (
            tc.tile_pool(name="pA", bufs=3) as pA,
            tc.tile_pool(name="pAp", bufs=2, space="PSUM") as pAp,
        ):
            for t in range(c.T):
                hp = pAp.tile([P, W1w], F32, tag="hp")
                for k in range(KC1):
                    _mm_cols(nc, hp, xT_sb[:, k * c.NLP + t * P:
                                           k * c.NLP + (t + 1) * P],
                             W1a_sb[:, k * W1w:(k + 1) * W1w], W1w,
                             start=(k == 0), stop=(k == KC1 - 1))
                _store_hs(nc, pA, hp, c.CO1, H, c.ELEM1, hs1_loc, d1t, t)

        if c.NCORES == 1:
            hs1_full, hs2_full, hs3_full = hs1_loc, hs2_loc, hs3_loc
        if max_phase >= 1 and c.NCORES > 1:
            nc.gpsimd.collective_compute(
                "AllGather", AL.bypass, replica_groups=groups,
                ins=[hs1_loc[:].opt()], outs=[hs1_full[:].opt()])

        # ================= layer-1 aggregation + LN + fused L2 matmul
        if max_phase >= 2:
            _edge_phase(
                nc, tc, c, meta, lay=1, Hn=H, Ch=c.C1, ELEM=c.ELEM1,
                hs_full=hs1_full, d_tab=d1t,
                brow_sb=(brow1_sb if meta.bias12 else None),
                grow_sb=(None if meta.gfold else grow1_sb),
                g3b3_sb=None,
                iota_sb=iota_sb, id_sb=id_sb, idx_sb=idx_sb, dl_sb=dl_sb,
                didx_sb=didx_sb,
                fuse=dict(W_sb=W2a_sb, KC=KC2, Ww=W2w, CO=c.CO2, Hn2=H,
                          ELEMn=c.ELEM2, hs_loc=hs2_loc, d_next=d2t),
                final=None, y=None)

        if max_phase >= 3 and c.NCORES > 1:
            nc.gpsimd.collective_compute(
                "AllGather", AL.bypass, replica_groups=groups,
                ins=[hs2_loc[:].opt()], outs=[hs2_full[:].opt()])

        # ================= layer-2 aggregation + LN + fused L3 matmul
        if max_phase >= 4:
            _edge_phase(
                nc, tc, c, meta, lay=2, Hn=H, Ch=c.C2, ELEM=c.ELEM2,
                hs_full=hs2_full, d_tab=d2t,
                brow_sb=(brow2_sb if meta.bias12 else None),
                grow_sb=(None if meta.gfold else grow2_sb),
                g3b3_sb=None,
                iota_sb=iota_sb, id_sb=id_sb, idx_sb=idx_sb, dl_sb=dl_sb,
                didx_sb=didx_sb,
                fuse=dict(W_sb=W3a_sb, KC=KC3, Ww=W3w, CO=c.NCLS, Hn2=1,
                          ELEMn=c.ELEM3, hs_loc=hs3_loc, d_next=d3t),
                final=None, y=None)

        if max_phase >= 5 and c.NCORES > 1:
            nc.gpsimd.collective_compute(
                "AllGather", AL.bypass, replica_groups=groups,
                ins=[hs3_loc[:].opt()], outs=[hs3_full[:].opt()])

        # ================= layer-3 aggregation + LN + log_softmax
        if max_phase >= 6:
            _edge_phase(
                nc, tc, c, meta, lay=3, Hn=1, Ch=c.NCLS, ELEM=c.ELEM3,
                hs_full=hs3_full, d_tab=d3t,
                brow_sb=None, grow_sb=None, g3b3_sb=g3b3_sb,
                iota_sb=iota_sb, id_sb=id_sb, idx_sb=idx_sb, dl_sb=dl_sb,
                didx_sb=didx_sb,
                fuse=None, final=True, y=y)

        cpool_cm.__exit__(None, None, None)
        dram_cm.__exit__(None, None, None)

    nc.compile()
    return nc


def _splits(W):
    out = []
    n0 = 0
    while n0 < W:
        nsz = min(512, W - n0)
        out.append((n0, nsz))
        n0 += nsz
    return out


def _mm_cols(nc, out_ps, lhsT, rhs, Ww, start, stop):
    """out_ps[:, :Ww] (+)= lhsT.T @ rhs, N split at 512."""
    for (n0, nsz) in _splits(Ww):
        nc.tensor.matmul(out=out_ps[:, n0:n0 + nsz], lhsT=lhsT,
                         rhs=rhs[:, n0:n0 + nsz], start=start, stop=stop)


def _mm_splits(nc, out_ps, lhs_sb, w_sb, KC, Ww, Plhs):
    """out_ps[:, :Ww] = sum_k lhs_k.T @ W_k, with N split at 512."""
    for k in range(KC):
        for (n0, nsz) in _splits(Ww):
            nc.tensor.matmul(
                out=out_ps[:, n0:n0 + nsz],
                lhsT=lhs_sb[:, k * Plhs:(k + 1) * Plhs],
                rhs=w_sb[:, k * Ww + n0:k * Ww + n0 + nsz],
                start=(k == 0), stop=(k == KC - 1))


def _store_hs(nc, pool, hp, CO, Hn, ELEM, hs_loc, d_tab, t):
    """PSUM [128, CO+2H] -> bf16 hs row tile + bf16 d table row tile."""
    hst = pool.tile([P, ELEM], BF16, tag="hst")
    nc.scalar.copy(hst[:, 0:CO], hp[:, 0:CO])
    nc.vector.tensor_copy(hst[:, CO:CO + Hn], hp[:, CO:CO + Hn])
    if ELEM > CO + Hn:
        nc.vector.memset(hst[:, CO + Hn:ELEM], 0)
    dt = pool.tile([P, P], BF16, tag="dt")
    nc.vector.tensor_copy(dt[:, 0:Hn], hp[:, CO + Hn:CO + 2 * Hn])
    nc.vector.memset(dt[:, Hn:P], 0)
    nc.sync.dma_start(hs_loc[t * P:(t + 1) * P, :], hst[:])
    nc.sync.dma_start(d_tab[t * P:(t + 1) * P, :], dt[:])


def _edge_phase(nc, tc, c: Cfg, meta: Meta, lay, Hn, Ch, ELEM, hs_full, d_tab,
                brow_sb, grow_sb, g3b3_sb, iota_sb, id_sb, idx_sb, dl_sb,
                didx_sb, fuse, final, y):
    CO = Hn * Ch
    max_nch = max(max(r) for r in meta.nch)
    max_ntot = max(r[0] + r[1] for r in meta.nch)
    use_den = not final           # L3: LN is invariant to the row scale
    AW = CO + (Hn if use_den else 0)   # Gw width (den cols at the tail)

    with (
        tc.tile_pool(name=f"sb{lay}", bufs=2) as sb,
        tc.tile_pool(name=f"sc{lay}", bufs=4) as sbc,
        tc.tile_pool(name=f"g{lay}", bufs=4) as gp,
        tc.tile_pool(name=f"gw{lay}", bufs=4) as gw,
        tc.tile_pool(name=f"ps{lay}", bufs=1, space="PSUM") as ps1,
        tc.tile_pool(name=f"pst{lay}", bufs=2, space="PSUM") as psT,
        tc.tile_pool(name=f"psagg{lay}", bufs=(1 if lay == 1 else 2),
                     space="PSUM") as psA,
    ):
        for t in range(c.T):
            nch0, nch1 = meta.nch[t]
            ntot = nch0 + nch1
            agg = psA.tile([P, CO], F32, tag="agg")
            den = (ps1.tile([P, Hn], F32, tag="den", name=f"den{lay}")
                   if use_den else None)
            if ntot == 0:
                nc.vector.memset(agg[:], 0)
                if use_den:
                    nc.vector.memset(den[:], 0)
                _epilogue(nc, sb, ps1, c, meta, lay, t, agg, den, Hn, Ch, CO,
                          use_den, brow_sb, grow_sb, g3b3_sb, id_sb,
                          fuse, final, y)
                continue

            # per-edge d rows for the tile (256B rows from the d table);
            # split per half to stay under the 64-descriptor packet limit
            Dg = gp.tile([P, max_ntot * P], BF16, tag="Dg")
            for hf, nch in ((0, nch0), (1, nch1)):
                if nch == 0:
                    continue
                b0 = 0 if hf == 0 else nch0
                sdo = meta.sd[t] + b0 * (P // 16)
                nc.gpsimd.dma_gather(
                    out_ap=Dg[:, b0 * P:(b0 + nch) * P].rearrange(
                        "p (k d) -> p k d", d=P),
                    in_ap=d_tab[:],
                    idxs_ap=didx_sb[:, sdo:sdo + nch * (P // 16)],
                    num_idxs=nch * P, num_idxs_reg=nch * P, elem_size=P)

            Gs = []
            for hf, nch in ((0, nch0), (1, nch1)):
                if nch == 0:
                    Gs.append(None)
                    continue
                G = gp.tile([P, max_nch * ELEM], BF16, tag="G")
                si = meta.si[t][hf]
                nidx = nch * P
                nc.gpsimd.dma_gather(
                    out_ap=G[:, 0:nch * ELEM].rearrange(
                        "p (k d) -> p k d", d=ELEM),
                    in_ap=hs_full[hf * c.HALF:(hf + 1) * c.HALF, :],
                    idxs_ap=idx_sb[:, si:si + nch * (P // 16)],
                    num_idxs=nidx, num_idxs_reg=nidx, elem_size=ELEM)
                Gs.append(G)

            # ---- per-tile prep: tsd = s_src + d_dst, leaky, exp
            tsda = sbc.tile([P, max_ntot * Hn], F32, tag="tsda")
            for hf, nch in ((0, nch0), (1, nch1)):
                if nch == 0:
                    continue
                b0 = 0 if hf == 0 else nch0
                Gv = Gs[hf][:, 0:nch * ELEM].rearrange(
                    "p (k d) -> p k d", d=ELEM)[:, :, CO:CO + Hn]
                Dv = Dg[:, b0 * P:(b0 + nch) * P].rearrange(
                    "p (k d) -> p k d", d=P)[:, :, 0:Hn]
                nc.vector.tensor_tensor(
                    out=tsda[:, b0 * Hn:(b0 + nch) * Hn].rearrange(
                        "p (k h) -> p k h", h=Hn),
                    in0=Gv, in1=Dv, op=AL.add)
            lra = sbc.tile([P, max_ntot * Hn], F32, tag="lra")
            nc.vector.scalar_tensor_tensor(
                out=lra[:, 0:ntot * Hn], in0=tsda[:, 0:ntot * Hn],
                scalar=NEG_SLOPE_ATT, in1=tsda[:, 0:ntot * Hn],
                op0=AL.mult, op1=AL.max)
            wfa = sbc.tile([P, max_ntot * Hn], F32, tag="wfa")
            nc.scalar.activation(wfa[:, 0:ntot * Hn], lra[:, 0:ntot * Hn],
                                 AF.Exp)
            if use_den:
                wfb = sbc.tile([P, max_ntot * Hn], BF16, tag="wfb")
                nc.vector.tensor_copy(wfb[:, 0:ntot * Hn],
                                      wfa[:, 0:ntot * Hn])

            # ---- one-hot columns for every chunk of the tile (bf16 lhsT)
            c0 = meta.sc[t][0]
            eqa = sbc.tile([P, max_ntot * P], BF16, tag="eqa")
            io = iota_sb[:]
            iob = bass.AP(io.tensor, io.offset,
                          [list(io.ap[0]), [0, ntot], list(io.ap[1])])
            nc.vector.tensor_tensor(
                out=eqa[:, 0:ntot * P].rearrange("p (k d) -> p k d", d=P),
                in0=iob, in1=dl_sb[:, c0:c0 + ntot].to_broadcast([P, ntot, P]),
                op=AL.is_equal)

            # ---- per chunk: alpha-scaled rhs (one 3D op), matmuls
            sp = _splits(CO)
            gchunk = 0
            for hf, nch in ((0, nch0), (1, nch1)):
                G = Gs[hf]
                for b in range(nch):
                    first = (gchunk == 0)
                    last = (gchunk == ntot - 1)
                    eq = eqa[:, gchunk * P:(gchunk + 1) * P]
                    Gw = gw.tile([P, CO], BF16, tag="Gw")
                    nc.vector.tensor_tensor(
                        out=Gw[:].rearrange("p (h c) -> p h c", h=Hn),
                        in0=G[:, b * ELEM:b * ELEM + CO].rearrange(
                            "p (h c) -> p h c", h=Hn),
                        in1=wfa[:, gchunk * Hn:(gchunk + 1) * Hn]
                            .to_broadcast([P, Hn, Ch]),
                        op=AL.mult)
                    for (n0, nsz) in sp:
                        nc.tensor.matmul(out=agg[:, n0:n0 + nsz], lhsT=eq,
                                         rhs=Gw[:, n0:n0 + nsz],
                                         start=first, stop=last)
                    if use_den:
                        nc.tensor.matmul(
                            out=den[:], lhsT=eq,
                            rhs=wfb[:, gchunk * Hn:(gchunk + 1) * Hn],
                            start=first, stop=last)
                    gchunk += 1

            _epilogue(nc, sb, ps1, psT, c, meta, lay, t, agg, den, Hn, Ch,
                      CO, use_den, brow_sb, grow_sb, g3b3_sb, id_sb,
                      fuse, final, y)


def _epilogue(nc, sb, ps1, psT, c, meta, lay, t, agg, den, Hn, Ch, CO,
              use_den, brow_sb, grow_sb, g3b3_sb, id_sb, fuse, final, y):
    # normalize by the softmax denominator (if needed), then LayerNorm with
    # gamma folded into the next weights; leaky + fused next-layer matmul,
    # or log_softmax for the final layer.
    ob = sb.tile([P, CO], BF16, tag="ob")
    rs = sb.tile([P, 1], F32, tag="rs")
    if use_den:
        denr = sb.tile([P, Hn], F32, tag="denr")
        nc.vector.tensor_scalar(out=denr[:], in0=den[:],
                                scalar1=1e-16, scalar2=None, op0=AL.add)
        rec = sb.tile([P, Hn], F32, tag="rec")
        nc.vector.reciprocal(rec[:], denr[:])
        nc.vector.tensor_tensor(
            out=ob[:].rearrange("p (h c) -> p h c", h=Hn),
            in0=agg[:].rearrange("p (h c) -> p h c", h=Hn),
            in1=rec[:].to_broadcast([P, Hn, Ch]), op=AL.mult)
        nc.vector.tensor_reduce(out=rs[:], in_=ob[:],
                                axis=mybir.AxisListType.X, op=AL.add)
    else:
        nc.vector.tensor_scalar(out=ob[:], in0=agg[:, 0:CO], scalar1=1.0,
                                scalar2=0.0, op0=AL.mult, op1=AL.add,
                                accum_out=rs[:])
    if brow_sb is not None:
        ob2 = sb.tile([P, CO], BF16, tag="ob2")
        nc.vector.tensor_tensor(out=ob2[:], in0=ob[:], in1=brow_sb[:],
                                op=AL.add)
        ob = ob2
        rs2 = sb.tile([P, 1], F32, tag="rs2")
        nc.vector.tensor_reduce(out=rs2[:], in_=ob[:],
                                axis=mybir.AxisListType.X, op=AL.add)
        rs = rs2
    # LayerNorm statistics: mean, then var = sum((x-mu)*x)/CO
    nm = sb.tile([P, 1], F32, tag="nm")
    nc.vector.tensor_scalar(out=nm[:], in0=rs[:], scalar1=1.0 / CO,
                            scalar2=None, op0=AL.mult)
    sqs = sb.tile([P, CO], BF16, tag="sqs")
    vs = sb.tile([P, 1], F32, tag="vs")
    nc.vector.scalar_tensor_tensor(
        out=sqs[:], in0=ob[:], scalar=nm[:, 0:1], in1=ob[:],
        op0=AL.subtract, op1=AL.mult, accum_out=vs[:])
    lnv = sb.tile([P, 1], F32, tag="lnv")
    nc.scalar.activation(lnv[:], vs[:], AF.Ln, bias=LN_EPS, scale=1.0 / CO)
    rstd = sb.tile([P, 1], F32, tag="rstd")
    nc.scalar.activation(rstd[:], lnv[:], AF.Exp, bias=0.0, scale=-0.5)
    y1 = sb.tile([P, CO], BF16, tag="y1")
    nc.vector.tensor_scalar(out=y1[:], in0=ob[:], scalar1=nm[:, 0:1],
                            scalar2=rstd[:, 0:1], op0=AL.subtract,
                            op1=AL.mult)
    if grow_sb is not None:
        yg = sb.tile([P, CO], BF16, tag="yg")
        nc.vector.tensor_tensor(out=yg[:], in0=y1[:], in1=grow_sb[:],
                                op=AL.mult)
        y1 = yg

    if final:
        # y2 = y1*g3 + b3, then log_softmax over CO, write y (f32 math)
        yg3 = sb.tile([P, CO], F32, tag="yg3")
        nc.vector.tensor_tensor(out=yg3[:], in0=y1[:],
                                in1=g3b3_sb[:, 0:CO], op=AL.mult)
        yb3 = sb.tile([P, CO], F32, tag="yb3")
        nc.vector.tensor_tensor(out=yb3[:], in0=yg3[:],
                                in1=g3b3_sb[:, CO:2 * CO], op=AL.add)
        mx = sb.tile([P, 1], F32, tag="mx")
        nc.vector.tensor_reduce(out=mx[:], in_=yb3[:],
                                axis=mybir.AxisListType.X, op=AL.max)
        xs = sb.tile([P, CO], F32, tag="xs")
        nc.vector.tensor_scalar(out=xs[:], in0=yb3[:], scalar1=mx[:, 0:1],
                                scalar2=None, op0=AL.subtract)
        ex = sb.tile([P, CO], F32, tag="ex")
        se = sb.tile([P, 1], F32, tag="se")
        nc.scalar.activation(ex[:], xs[:], AF.Exp, accum_out=se[:])
        lse = sb.tile([P, 1], F32, tag="lse")
        nc.scalar.activation(lse[:], se[:], AF.Ln)
        yo = sb.tile([P, CO], F32, tag="yo")
        nc.vector.tensor_scalar(out=yo[:], in0=xs[:], scalar1=lse[:, 0:1],
                                scalar2=None, op0=AL.subtract)
        nc.sync.dma_start(y[t * P:(t + 1) * P, :], yo[:])
        return

    # leaky(0.01) -> bf16 x_next; fused next-layer matmul
    x2 = sb.tile([P, CO], BF16, tag="x2")
    nc.vector.scalar_tensor_tensor(
        out=x2[:], in0=y1[:], scalar=NEG_SLOPE_ACT, in1=y1[:],
        op0=AL.mult, op1=AL.max)
    W_sb, KC, Ww = fuse["W_sb"], fuse["KC"], fuse["Ww"]
    CO2, Hn2, ELEMn = fuse["CO"], fuse["Hn2"], fuse["ELEMn"]
    xt2 = sb.tile([P, KC * P], BF16, tag="xt2")
    for k in range(KC):
        scr = psT.tile([P, P], BF16, tag="scr")
        nc.tensor.transpose(out=scr[:], in_=x2[:, k * P:(k + 1) * P],
                            identity=id_sb[:])
        nc.scalar.copy(xt2[:, k * P:(k + 1) * P], scr[:])
    hp = ps1.tile([P, Ww], F32, tag="hnext")
    _mm_splits(nc, hp, xt2, W_sb, KC, Ww, P)
    _store_hs(nc, sb, hp, CO2, Hn2, ELEMn, fuse["hs_loc"], fuse["d_next"], t)


# --------------------------------------------------------------------------
# entry point
# --------------------------------------------------------------------------

_CACHE = {}


def _get_nc(cfg, meta):
    key = (tuple(sorted(cfg.__dict__.items())),
           tuple(tuple(r) for r in meta.nch), meta.bias12, meta.gfold)
    if key not in _CACHE:
        _CACHE[key] = build_nc(cfg, meta)
    return _CACHE[key]


def kernel(**inputs):
    inputs = {k: np.asarray(v) for k, v in inputs.items()}
    x = inputs["x"]
    cfg = Cfg(N=x.shape[0], E=inputs["edge_src"].shape[0], F_IN=x.shape[1],
              HEADS=inputs["a_src1"].shape[0], C1=inputs["a_src1"].shape[1],
              C2=inputs["a_src2"].shape[1], NCLS=inputs["W3"].shape[0],
              NCORES=8)
    in_maps, meta = host_prep(cfg, **inputs)
    nc = _get_nc(cfg, meta)
    trace = bool(int(os.environ.get("GAT_TRACE", "0")))
    res = run_bass_kernel_spmd(nc, in_maps, core_ids=list(range(cfg.NCORES)),
                               trace=trace)
    global LAST_EXEC_NS
    LAST_EXEC_NS = res.exec_time_ns
    out = np.concatenate(
        [res.results[cc]["y"][:cfg.NL] for cc in range(cfg.NCORES)], axis=0)
    return out.astype(np.float32)


LAST_EXEC_NS = None


if __name__ == "__main__":
    pass


# revision 14
# speedup vs baseline: 1.4401x; 1.1426x over previous
"""3-layer GAT on 8 Trainium2 NeuronCores (Bass/Tile).

Strategy: partition nodes across the 8 cores (graph parallel); edges live with
their destination core so segment-softmax/aggregation stay local; per layer,
all-gather the (bf16) node features + attention source logits; gather source
rows per edge chunk with dma_gather; aggregate with one-hot selection matmuls
on the PE (single ldweights per 128-edge chunk, alpha applied to the gathered
rows on the DVE in fast mode, softmax denominator folded into the same rhs).

LayerNorm gammas are folded into the next layer's weights on the host
(leaky_relu is positive-homogeneous); zero biases skip the bias add; the
layer-3 softmax-denominator division is skipped entirely (LN scale
invariance).  The only activation functions used on the ACT engine are
{Exp, Ln, Copy}, which share one table set - no act-table reloads.

Self-contained: only imports the system concourse install.
"""

import os
import sys

for _p in ("/opt/trn_rl_repo", "/root/.axon_site/_ro/trn_rl_repo"):
    if os.path.isdir(_p) and _p not in sys.path:
        sys.path.insert(0, _p)

from dataclasses import dataclass

import ml_dtypes
import numpy as np

import concourse.bacc as bacc
import concourse.bass as bass
import concourse.tile as tile
from concourse import mybir
from concourse.bass_utils import run_bass_kernel_spmd

_orig_get_act_tables = bacc.get_activation_tables


def _patched_act_tables(arch):
    tabs = dict(_orig_get_act_tables(arch))
    AFT = mybir.ActivationFunctionType
    combined = None
    for name, fns in tabs.items():
        if AFT.Exp in fns and AFT.Ln in fns:
            combined = name
            break
    if combined is not None:
        for name in list(tabs):
            if name != combined:
                tabs[name] = tabs[name] - {AFT.Exp, AFT.Ln}
    return tabs


bacc.get_activation_tables = _patched_act_tables

P = 128
BF16 = mybir.dt.bfloat16
F32 = mybir.dt.float32
I16 = mybir.dt.int16
AL = mybir.AluOpType
AF = mybir.ActivationFunctionType

NEG_SLOPE_ATT = 0.2
NEG_SLOPE_ACT = 0.01
LN_EPS = 1e-5


def _ceil(a, b):
    return -(-a // b)


def _ceil_arr(a, b):
    return -(-a // b)


def _pad_elem(n_f32_elems):
    """bf16 row length (elements) padded so row bytes are a multiple of 256."""
    return _ceil(n_f32_elems * 2, 256) * 128


@dataclass
class Cfg:
    N: int = 50000
    E: int = 400000
    F_IN: int = 256
    HEADS: int = 4
    C1: int = 256
    C2: int = 128
    NCLS: int = 32
    NCORES: int = 8

    def __post_init__(self):
        assert self.N % self.NCORES == 0
        self.NL = self.N // self.NCORES
        self.T = _ceil(self.NL, P)
        self.NLP = self.T * P
        self.NPTOT = self.NLP * self.NCORES
        assert self.NPTOT % 2 == 0
        self.HALF = self.NPTOT // 2
        assert self.HALF <= 32767, "half-table must be int16 addressable"
        H = self.HEADS
        self.CO1 = H * self.C1
        self.CO2 = H * self.C2
        assert self.F_IN % P == 0 and self.CO1 % P == 0 and self.CO2 % P == 0
        # bf16 hs-table rows: [h | s | pad], bytes % 256 == 0
        self.ELEM1 = _pad_elem(self.CO1 + H)
        self.ELEM2 = _pad_elem(self.CO2 + H)
        self.ELEM3 = _pad_elem(self.NCLS + 1)


@dataclass
class Meta:
    nch: list  # [T][2] chunk counts (common across cores)
    si: list   # [T][2] idx16 column offsets
    sc: list   # [T][2] dstloc column offsets
    SI: int
    SC: int
    sd: list = None   # [T] dst-idx column offsets (per-tile d gather)
    SD: int = 0
    bias12: bool = False  # layer-1/2 biases nonzero -> add brow in epilogue
    gfold: bool = True    # gammas folded into next-layer weights


def host_prep(cfg: Cfg, x, edge_src, edge_dst,
              W1, a_src1, a_dst1, b1, ln1_g, ln1_b,
              W2, a_src2, a_dst2, b2, ln2_g, ln2_b,
              W3, a_src3, a_dst3, b3, ln3_g, ln3_b):
    """Build per-core input maps + the (common) chunk structure."""
    c = cfg
    bf = ml_dtypes.bfloat16

    # ---- append self loops, shard edges by destination core
    loops = np.arange(c.N, dtype=np.int64)
    src = np.concatenate([edge_src.astype(np.int64), loops])
    dst = np.concatenate([edge_dst.astype(np.int64), loops])

    dst_core = dst // c.NL
    dstloc = dst - dst_core * c.NL
    tile_id = dstloc // P
    gsrc = (src // c.NL) * c.NLP + (src % c.NL)     # padded-global source row
    half = (gsrc >= c.HALF).astype(np.int64)
    idx16 = (gsrc - half * c.HALF).astype(np.int64)

    # group edges per (core, tile, half)
    counts = np.zeros((c.NCORES, c.T, 2), np.int64)
    np.add.at(counts, (dst_core, tile_id, half), 1)
    nch = np.maximum(_ceil_arr(counts.max(axis=0), P), 0)  # [T,2] chunks
    # offsets
    si = np.zeros((c.T, 2), np.int64)
    sc = np.zeros((c.T, 2), np.int64)
    acc_si = acc_sc = 0
    for t in range(c.T):
        for h in range(2):
            si[t, h] = acc_si
            sc[t, h] = acc_sc
            acc_si += int(nch[t, h]) * (P // 16)
            acc_sc += int(nch[t, h])
    SI, SC = int(acc_si), int(acc_sc)
    sd = np.zeros(c.T, np.int64)
    acc_sd = 0
    for t in range(c.T):
        sd[t] = acc_sd
        acc_sd += int(nch[t, 0] + nch[t, 1]) * (P // 16)
    SD = int(acc_sd)

    bias12 = bool(np.any(b1) or np.any(b2))
    gfold = bool(np.all(ln1_g > 0) and np.all(ln2_g > 0))
    meta = Meta(nch=nch.tolist(), si=si.tolist(), sc=sc.tolist(), SI=SI, SC=SC,
                sd=sd.tolist(), SD=SD, bias12=bias12, gfold=gfold)

    # ---- per-core index / dstloc tables
    order = np.lexsort((half, tile_id, dst_core))  # group by (core, tile, half)
    src_s, half_s, t_s, core_s = (idx16[order], half[order], tile_id[order],
                                  dst_core[order])
    dstrel_s = (dstloc - tile_id * P)[order]

    idx_tabs, dl_tabs = [], []
    # per-core group start offsets
    starts = np.zeros((c.NCORES, c.T, 2), np.int64)
    run = 0
    for cc in range(c.NCORES):
        for t in range(c.T):
            for h in range(2):
                starts[cc, t, h] = run
                run += int(counts[cc, t, h])
    dstidx_tabs = []
    cnt_tabs = []
    for cc in range(c.NCORES):
        gcnt = np.zeros((1, 2 * c.T), np.int32)
        for t in range(c.T):
            for h in range(2):
                m = int(counts[cc, t, h])
                gcnt[0, 2 * t + h] = max(_ceil(m, 16) * 16, 16)
        cnt_tabs.append(gcnt)
        itab = np.zeros((16, SI), np.int16)
        dtab = np.full((P, SC), -1.0, np.float32)
        ditab = np.zeros((16, SD), np.int16)
        for t in range(c.T):
            dchunk = 0  # chunk index within the tile (across halves)
            for h in range(2):
                m = int(counts[cc, t, h])
                n = int(nch[t, h])
                if n == 0:
                    continue
                s0 = int(starts[cc, t, h])
                iv = np.zeros(n * P, np.int16)
                iv[:m] = src_s[s0:s0 + m].astype(np.int16)
                cols = int(si[t, h])
                blk = iv.reshape(n * P // 16, 16).T  # idx k -> [k%16, k//16]
                itab[:, cols:cols + n * (P // 16)] = blk
                dv = np.full(n * P, -1.0, np.float32)
                dv[:m] = dstrel_s[s0:s0 + m].astype(np.float32)
                dtab[:, sc[t, h]:sc[t, h] + n] = dv.reshape(n, P).T
                # dst-row indices (into the local [NLP] d table)
                div = np.zeros(n * P, np.int16)
                div[:m] = (t * P + dstrel_s[s0:s0 + m]).astype(np.int16)
                dc = int(sd[t]) + dchunk * (P // 16)
                ditab[:, dc:dc + n * (P // 16)] = div.reshape(
                    n * P // 16, 16).T
                dchunk += n
        idx_tabs.append(np.tile(itab, (8, 1)))
        dl_tabs.append(dtab)
        dstidx_tabs.append(np.tile(ditab, (8, 1)))

    # ---- weights (augmented with U = W.T @ a columns), bf16
    def aug(W, a_s, a_d, H, C, g_in=None):
        WT = W.T.astype(np.float64)                      # [Fin, H*C]
        U_s = np.zeros((WT.shape[0], H))
        U_d = np.zeros((WT.shape[0], H))
        for h in range(H):
            U_s[:, h] = WT[:, h * C:(h + 1) * C] @ a_s[h].astype(np.float64)
            U_d[:, h] = WT[:, h * C:(h + 1) * C] @ a_d[h].astype(np.float64)
        out = np.concatenate([WT, U_s, U_d], axis=1)
        if g_in is not None:
            out = out * g_in.astype(np.float64)[:, None]
        return out.astype(bf)

    g1 = ln1_g if gfold else None
    g2 = ln2_g if gfold else None
    W1a = aug(W1, a_src1, a_dst1, c.HEADS, c.C1)           # [F_IN, CO1+2H]
    W2a = aug(W2, a_src2, a_dst2, c.HEADS, c.C2, g1)       # [CO1, CO2+2H]
    W3a = aug(W3, a_src3, a_dst3, 1, c.NCLS, g2)           # [CO2, NCLS+2]

    def row(v):
        return np.repeat(np.asarray(v, np.float32)[None, :].astype(bf),
                         P, axis=0)

    brow1 = row(b1)
    brow2 = row(b2)
    grow1 = row(ln1_g)
    grow2 = row(ln2_g)
    g3b3 = np.concatenate([row(ln3_g), row(b3)], axis=1)   # [128, 2*NCLS]

    iota_b = np.repeat(np.arange(P, dtype=np.float32)[None, :],
                       P, axis=0).astype(bf)
    ident = np.eye(P, dtype=bf)

    in_maps = []
    for cc in range(c.NCORES):
        xl = np.zeros((c.NLP, c.F_IN), np.float32)
        xl[:c.NL] = x[cc * c.NL:(cc + 1) * c.NL]
        in_maps.append({
            "xT": np.ascontiguousarray(xl.T).astype(bf),
            "W1a": W1a, "W2a": W2a, "W3a": W3a,
            "brow1": brow1, "brow2": brow2,
            "grow1": grow1, "grow2": grow2, "g3b3": g3b3,
            "idx16": idx_tabs[cc], "dstloc": dl_tabs[cc],
            "dstidx16": dstidx_tabs[cc], "gcnt": cnt_tabs[cc],
            "iotab": iota_b, "ident": ident,
        })
    return in_maps, meta


# --------------------------------------------------------------------------
# device program
# --------------------------------------------------------------------------

def build_nc(cfg: Cfg, meta: Meta, max_phase: int = 6):
    c = cfg
    H = c.HEADS
    nc = bacc.Bacc("TRN2", target_bir_lowering=False, debug=False,
                   num_devices=c.NCORES, enable_partition_id=False)

    eps_t = nc.alloc_sbuf_tensor("const-f32-lneps", [128, 1], F32)
    nc.gpsimd.memset(eps_t.ap(), LN_EPS)
    nc.const_aps.aps[(F32, LN_EPS)] = eps_t.ap()

    # ---- I/O
    xT = nc.dram_tensor("xT", [c.F_IN, c.NLP], BF16, kind="ExternalInput").ap()
    W1a = nc.dram_tensor("W1a", [c.F_IN, c.CO1 + 2 * H], BF16, kind="ExternalInput").ap()
    W2a = nc.dram_tensor("W2a", [c.CO1, c.CO2 + 2 * H], BF16, kind="ExternalInput").ap()
    W3a = nc.dram_tensor("W3a", [c.CO2, c.NCLS + 2], BF16, kind="ExternalInput").ap()
    brow1 = nc.dram_tensor("brow1", [P, c.CO1], BF16, kind="ExternalInput").ap()
    brow2 = nc.dram_tensor("brow2", [P, c.CO2], BF16, kind="ExternalInput").ap()
    grow1 = nc.dram_tensor("grow1", [P, c.CO1], BF16, kind="ExternalInput").ap()
    grow2 = nc.dram_tensor("grow2", [P, c.CO2], BF16, kind="ExternalInput").ap()
    g3b3 = nc.dram_tensor("g3b3", [P, 2 * c.NCLS], BF16, kind="ExternalInput").ap()
    idx16 = nc.dram_tensor("idx16", [P, meta.SI], I16, kind="ExternalInput").ap()
    dstloc = nc.dram_tensor("dstloc", [P, meta.SC], F32, kind="ExternalInput").ap()
    dstidx16 = nc.dram_tensor("dstidx16", [P, meta.SD], I16,
                              kind="ExternalInput").ap()
    gcnt = nc.dram_tensor("gcnt", [1, 2 * c.T], mybir.dt.int32,
                          kind="ExternalInput").ap()
    iotab = nc.dram_tensor("iotab", [P, P], BF16, kind="ExternalInput").ap()
    ident = nc.dram_tensor("ident", [P, P], BF16, kind="ExternalInput").ap()
    y = nc.dram_tensor("y", [c.NLP, c.NCLS], F32, kind="ExternalOutput").ap()

    groups = [list(range(c.NCORES))]

    with tile.TileContext(nc) as tc:
        # ---- persistent DRAM intermediates
        dram_cm = tc.tile_pool(name="dram", bufs=1, space="DRAM")
        dram = dram_cm.__enter__()
        aspace = "Shared" if c.NCORES > 4 else "Local"
        hs1_loc = dram.tile([c.NLP, c.ELEM1], BF16)
        hs1_full = dram.tile([c.NPTOT, c.ELEM1], BF16, addr_space=aspace)
        hs2_loc = dram.tile([c.NLP, c.ELEM2], BF16)
        hs2_full = dram.tile([c.NPTOT, c.ELEM2], BF16, addr_space=aspace)
        hs3_loc = dram.tile([c.NLP, c.ELEM3], BF16)
        hs3_full = dram.tile([c.NPTOT, c.ELEM3], BF16, addr_space=aspace)
        # d tables with 256B rows for dma_gather (cols 0:H hold d, rest pad)
        d1t = dram.tile([c.NLP, P], BF16)
        d2t = dram.tile([c.NLP, P], BF16)
        d3t = dram.tile([c.NLP, P], BF16)

        # ---- persistent SBUF constants
        cpool_cm = tc.tile_pool(name="const", bufs=1)
        cpool = cpool_cm.__enter__()
        KC1 = c.F_IN // P
        xT_sb = cpool.tile([P, KC1 * c.NLP], BF16)
        for k in range(KC1):
            nc.sync.dma_start(xT_sb[:, k * c.NLP:(k + 1) * c.NLP],
                              xT[k * P:(k + 1) * P, :])
        W1w = c.CO1 + 2 * H
        W1a_sb = cpool.tile([P, KC1 * W1w], BF16)
        for k in range(KC1):
            nc.sync.dma_start(W1a_sb[:, k * W1w:(k + 1) * W1w],
                              W1a[k * P:(k + 1) * P, :])
        KC2 = c.CO1 // P
        W2w = c.CO2 + 2 * H
        W2a_sb = cpool.tile([P, KC2 * W2w], BF16)
        for k in range(KC2):
            nc.sync.dma_start(W2a_sb[:, k * W2w:(k + 1) * W2w],
                              W2a[k * P:(k + 1) * P, :])
        KC3 = c.CO2 // P
        W3w = c.NCLS + 2
        W3a_sb = cpool.tile([P, KC3 * W3w], BF16)
        for k in range(KC3):
            nc.sync.dma_start(W3a_sb[:, k * W3w:(k + 1) * W3w],
                              W3a[k * P:(k + 1) * P, :])
        brow1_sb = cpool.tile([P, c.CO1], BF16)
        nc.sync.dma_start(brow1_sb[:], brow1[:])
        brow2_sb = cpool.tile([P, c.CO2], BF16)
        nc.sync.dma_start(brow2_sb[:], brow2[:])
        grow1_sb = cpool.tile([P, c.CO1], BF16)
        nc.sync.dma_start(grow1_sb[:], grow1[:])
        grow2_sb = cpool.tile([P, c.CO2], BF16)
        nc.sync.dma_start(grow2_sb[:], grow2[:])
        g3b3_sb = cpool.tile([P, 2 * c.NCLS], BF16)
        nc.sync.dma_start(g3b3_sb[:], g3b3[:])
        idx_sb = cpool.tile([P, meta.SI], I16)
        nc.sync.dma_start(idx_sb[:], idx16[:])
        dl_sb = cpool.tile([P, meta.SC], F32)
        nc.sync.dma_start(dl_sb[:], dstloc[:])
        didx_sb = cpool.tile([P, meta.SD], I16)
        nc.sync.dma_start(didx_sb[:], dstidx16[:])
        gcnt_sb = cpool.tile([1, 2 * c.T], mybir.dt.int32)
        nc.sync.dma_start(gcnt_sb[:], gcnt[:])
        iota_sb = cpool.tile([P, P], BF16)
        nc.sync.dma_start(iota_sb[:], iotab[:])
        id_sb = cpool.tile([P, P], BF16)
        nc.sync.dma_start(id_sb[:], ident[:])

        # ================= phase A: h1 = x @ W1a (per local node tile)
        with (
            tc.tile_pool(name="pA", bufs=3) as pA,
            tc.tile_pool(name="pAp", bufs=2, space="PSUM") as pAp,
        ):
            for t in range(c.T):
                hp = pAp.tile([P, W1w], F32, tag="hp")
                for k in range(KC1):
                    _mm_cols(nc, hp, xT_sb[:, k * c.NLP + t * P:
                                           k * c.NLP + (t + 1) * P],
                             W1a_sb[:, k * W1w:(k + 1) * W1w], W1w,
                             start=(k == 0), stop=(k == KC1 - 1))
                _store_hs(nc, pA, hp, c.CO1, H, c.ELEM1, hs1_loc, d1t, t)

        if c.NCORES == 1:
            hs1_full, hs2_full, hs3_full = hs1_loc, hs2_loc, hs3_loc
        if max_phase >= 1 and c.NCORES > 1:
            nc.gpsimd.collective_compute(
                "AllGather", AL.bypass, replica_groups=groups,
                ins=[hs1_loc[:].opt()], outs=[hs1_full[:].opt()])

        # ================= layer-1 aggregation + LN + fused L2 matmul
        if max_phase >= 2:
            _edge_phase(
                nc, tc, c, meta, lay=1, Hn=H, Ch=c.C1, ELEM=c.ELEM1,
                hs_full=hs1_full, d_tab=d1t,
                brow_sb=(brow1_sb if meta.bias12 else None),
                grow_sb=(None if meta.gfold else grow1_sb),
                g3b3_sb=None,
                iota_sb=iota_sb, id_sb=id_sb, idx_sb=idx_sb, dl_sb=dl_sb,
                didx_sb=didx_sb, gcnt_sb=gcnt_sb,
                fuse=dict(W_sb=W2a_sb, KC=KC2, Ww=W2w, CO=c.CO2, Hn2=H,
                          ELEMn=c.ELEM2, hs_loc=hs2_loc, d_next=d2t),
                final=None, y=None)

        if max_phase >= 3 and c.NCORES > 1:
            nc.gpsimd.collective_compute(
                "AllGather", AL.bypass, replica_groups=groups,
                ins=[hs2_loc[:].opt()], outs=[hs2_full[:].opt()])

        # ================= layer-2 aggregation + LN + fused L3 matmul
        if max_phase >= 4:
            _edge_phase(
                nc, tc, c, meta, lay=2, Hn=H, Ch=c.C2, ELEM=c.ELEM2,
                hs_full=hs2_full, d_tab=d2t,
                brow_sb=(brow2_sb if meta.bias12 else None),
                grow_sb=(None if meta.gfold else grow2_sb),
                g3b3_sb=None,
                iota_sb=iota_sb, id_sb=id_sb, idx_sb=idx_sb, dl_sb=dl_sb,
                didx_sb=didx_sb, gcnt_sb=gcnt_sb,
                fuse=dict(W_sb=W3a_sb, KC=KC3, Ww=W3w, CO=c.NCLS, Hn2=1,
                          ELEMn=c.ELEM3, hs_loc=hs3_loc, d_next=d3t),
                final=None, y=None)

        if max_phase >= 5 and c.NCORES > 1:
            nc.gpsimd.collective_compute(
                "AllGather", AL.bypass, replica_groups=groups,
                ins=[hs3_loc[:].opt()], outs=[hs3_full[:].opt()])

        # ================= layer-3 aggregation + LN + log_softmax
        if max_phase >= 6:
            _edge_phase(
                nc, tc, c, meta, lay=3, Hn=1, Ch=c.NCLS, ELEM=c.ELEM3,
                hs_full=hs3_full, d_tab=d3t,
                brow_sb=None, grow_sb=None, g3b3_sb=g3b3_sb,
                iota_sb=iota_sb, id_sb=id_sb, idx_sb=idx_sb, dl_sb=dl_sb,
                didx_sb=didx_sb, gcnt_sb=gcnt_sb,
                fuse=None, final=True, y=y)

        cpool_cm.__exit__(None, None, None)
        dram_cm.__exit__(None, None, None)

    nc.compile()
    return nc


def _splits(W):
    out = []
    n0 = 0
    while n0 < W:
        nsz = min(512, W - n0)
        out.append((n0, nsz))
        n0 += nsz
    return out


def _mm_cols(nc, out_ps, lhsT, rhs, Ww, start, stop):
    """out_ps[:, :Ww] (+)= lhsT.T @ rhs, N split at 512."""
    for (n0, nsz) in _splits(Ww):
        nc.tensor.matmul(out=out_ps[:, n0:n0 + nsz], lhsT=lhsT,
                         rhs=rhs[:, n0:n0 + nsz], start=start, stop=stop)


def _mm_splits(nc, out_ps, lhs_sb, w_sb, KC, Ww, Plhs):
    """out_ps[:, :Ww] = sum_k lhs_k.T @ W_k, with N split at 512."""
    for k in range(KC):
        for (n0, nsz) in _splits(Ww):
            nc.tensor.matmul(
                out=out_ps[:, n0:n0 + nsz],
                lhsT=lhs_sb[:, k * Plhs:(k + 1) * Plhs],
                rhs=w_sb[:, k * Ww + n0:k * Ww + n0 + nsz],
                start=(k == 0), stop=(k == KC - 1))


def _store_hs(nc, pool, hp, CO, Hn, ELEM, hs_loc, d_tab, t):
    """PSUM [128, CO+2H] -> bf16 hs row tile + bf16 d table row tile."""
    hst = pool.tile([P, ELEM], BF16, tag="hst")
    nc.scalar.copy(hst[:, 0:CO], hp[:, 0:CO])
    nc.vector.tensor_copy(hst[:, CO:CO + Hn], hp[:, CO:CO + Hn])
    if ELEM > CO + Hn:
        nc.vector.memset(hst[:, CO + Hn:ELEM], 0)
    dt = pool.tile([P, P], BF16, tag="dt")
    nc.vector.tensor_copy(dt[:, 0:Hn], hp[:, CO + Hn:CO + 2 * Hn])
    nc.vector.memset(dt[:, Hn:P], 0)
    nc.sync.dma_start(hs_loc[t * P:(t + 1) * P, :], hst[:])
    nc.sync.dma_start(d_tab[t * P:(t + 1) * P, :], dt[:])


def _edge_phase(nc, tc, c: Cfg, meta: Meta, lay, Hn, Ch, ELEM, hs_full, d_tab,
                brow_sb, grow_sb, g3b3_sb, iota_sb, id_sb, idx_sb, dl_sb,
                didx_sb, gcnt_sb, fuse, final, y):
    CO = Hn * Ch
    max_nch = max(max(r) for r in meta.nch)
    max_ntot = max(r[0] + r[1] for r in meta.nch)
    use_den = not final           # L3: LN is invariant to the row scale
    AW = CO + (Hn if use_den else 0)   # Gw width (den cols at the tail)

    creg = nc.gpsimd.alloc_register(f"gcnt_reg_{lay}")
    with (
        tc.tile_pool(name=f"sb{lay}", bufs=2) as sb,
        tc.tile_pool(name=f"sc{lay}", bufs=4) as sbc,
        tc.tile_pool(name=f"g{lay}", bufs=4) as gp,
        tc.tile_pool(name=f"gw{lay}", bufs=4) as gw,
        tc.tile_pool(name=f"ps{lay}", bufs=1, space="PSUM") as ps1,
        tc.tile_pool(name=f"pst{lay}", bufs=2, space="PSUM") as psT,
        tc.tile_pool(name=f"psagg{lay}", bufs=(1 if lay == 1 else 2),
                     space="PSUM") as psA,
    ):
        for t in range(c.T):
            nch0, nch1 = meta.nch[t]
            ntot = nch0 + nch1
            agg = psA.tile([P, CO], F32, tag="agg")
            den = (ps1.tile([P, Hn], F32, tag="den", name=f"den{lay}")
                   if use_den else None)
            if ntot == 0:
                nc.vector.memset(agg[:], 0)
                if use_den:
                    nc.vector.memset(den[:], 0)
                _epilogue(nc, sb, ps1, c, meta, lay, t, agg, den, Hn, Ch, CO,
                          use_den, brow_sb, grow_sb, g3b3_sb, id_sb,
                          fuse, final, y)
                continue

            # per-edge d rows for the tile (256B rows from the d table);
            # split per half to stay under the 64-descriptor packet limit
            Dg = gp.tile([P, max_ntot * P], BF16, tag="Dg")
            dyn = False  # dynamic counts deadlock the DMA completion sems
            for hf, nch in ((0, nch0), (1, nch1)):
                if nch == 0:
                    continue
                b0 = 0 if hf == 0 else nch0
                sdo = meta.sd[t] + b0 * (P // 16)
                if dyn:
                    nc.gpsimd.reg_load(
                        creg, gcnt_sb[0:1, 2 * t + hf:2 * t + hf + 1])
                nc.gpsimd.dma_gather(
                    out_ap=Dg[:, b0 * P:(b0 + nch) * P].rearrange(
                        "p (k d) -> p k d", d=P),
                    in_ap=d_tab[:],
                    idxs_ap=didx_sb[:, sdo:sdo + nch * (P // 16)],
                    num_idxs=nch * P,
                    num_idxs_reg=(creg if dyn else nch * P), elem_size=P)

            Gs = []
            for hf, nch in ((0, nch0), (1, nch1)):
                if nch == 0:
                    Gs.append(None)
                    continue
                G = gp.tile([P, max_nch * ELEM], BF16, tag="G")
                si = meta.si[t][hf]
                nidx = nch * P
                if dyn:
                    nc.gpsimd.reg_load(
                        creg, gcnt_sb[0:1, 2 * t + hf:2 * t + hf + 1])
                nc.gpsimd.dma_gather(
                    out_ap=G[:, 0:nch * ELEM].rearrange(
                        "p (k d) -> p k d", d=ELEM),
                    in_ap=hs_full[hf * c.HALF:(hf + 1) * c.HALF, :],
                    idxs_ap=idx_sb[:, si:si + nch * (P // 16)],
                    num_idxs=nidx,
                    num_idxs_reg=(creg if dyn else nidx), elem_size=ELEM)
                Gs.append(G)

            # ---- per-tile prep: tsd = s_src + d_dst, leaky, exp
            tsda = sbc.tile([P, max_ntot * Hn], F32, tag="tsda")
            for hf, nch in ((0, nch0), (1, nch1)):
                if nch == 0:
                    continue
                b0 = 0 if hf == 0 else nch0
                Gv = Gs[hf][:, 0:nch * ELEM].rearrange(
                    "p (k d) -> p k d", d=ELEM)[:, :, CO:CO + Hn]
                Dv = Dg[:, b0 * P:(b0 + nch) * P].rearrange(
                    "p (k d) -> p k d", d=P)[:, :, 0:Hn]
                nc.vector.tensor_tensor(
                    out=tsda[:, b0 * Hn:(b0 + nch) * Hn].rearrange(
                        "p (k h) -> p k h", h=Hn),
                    in0=Gv, in1=Dv, op=AL.add)
            lra = sbc.tile([P, max_ntot * Hn], F32, tag="lra")
            nc.vector.scalar_tensor_tensor(
                out=lra[:, 0:ntot * Hn], in0=tsda[:, 0:ntot * Hn],
                scalar=NEG_SLOPE_ATT, in1=tsda[:, 0:ntot * Hn],
                op0=AL.mult, op1=AL.max)
            wfa = sbc.tile([P, max_ntot * Hn], F32, tag="wfa")
            nc.scalar.activation(wfa[:, 0:ntot * Hn], lra[:, 0:ntot * Hn],
                                 AF.Exp)
            if use_den:
                wfb = sbc.tile([P, max_ntot * Hn], BF16, tag="wfb")
                nc.vector.tensor_copy(wfb[:, 0:ntot * Hn],
                                      wfa[:, 0:ntot * Hn])

            # ---- one-hot columns for every chunk of the tile (bf16 lhsT)
            c0 = meta.sc[t][0]
            eqa = sbc.tile([P, max_ntot * P], BF16, tag="eqa")
            io = iota_sb[:]
            iob = bass.AP(io.tensor, io.offset,
                          [list(io.ap[0]), [0, ntot], list(io.ap[1])])
            nc.vector.tensor_tensor(
                out=eqa[:, 0:ntot * P].rearrange("p (k d) -> p k d", d=P),
                in0=iob, in1=dl_sb[:, c0:c0 + ntot].to_broadcast([P, ntot, P]),
                op=AL.is_equal)

            # ---- per chunk: alpha-scaled rhs (one 3D op), matmuls
            sp = _splits(CO)
            gchunk = 0
            for hf, nch in ((0, nch0), (1, nch1)):
                G = Gs[hf]
                for b in range(nch):
                    first = (gchunk == 0)
                    last = (gchunk == ntot - 1)
                    eq = eqa[:, gchunk * P:(gchunk + 1) * P]
                    Gw = gw.tile([P, CO], BF16, tag="Gw")
                    nc.vector.tensor_tensor(
                        out=Gw[:].rearrange("p (h c) -> p h c", h=Hn),
                        in0=G[:, b * ELEM:b * ELEM + CO].rearrange(
                            "p (h c) -> p h c", h=Hn),
                        in1=wfa[:, gchunk * Hn:(gchunk + 1) * Hn]
                            .to_broadcast([P, Hn, Ch]),
                        op=AL.mult)
                    for (n0, nsz) in sp:
                        nc.tensor.matmul(out=agg[:, n0:n0 + nsz], lhsT=eq,
                                         rhs=Gw[:, n0:n0 + nsz],
                                         start=first, stop=last)
                    if use_den:
                        nc.tensor.matmul(
                            out=den[:], lhsT=eq,
                            rhs=wfb[:, gchunk * Hn:(gchunk + 1) * Hn],
                            start=first, stop=last)
                    gchunk += 1

            _epilogue(nc, sb, ps1, psT, c, meta, lay, t, agg, den, Hn, Ch,
                      CO, use_den, brow_sb, grow_sb, g3b3_sb, id_sb,
                      fuse, final, y)


def _epilogue(nc, sb, ps1, psT, c, meta, lay, t, agg, den, Hn, Ch, CO,
              use_den, brow_sb, grow_sb, g3b3_sb, id_sb, fuse, final, y):
    # normalize by the softmax denominator (if needed), then LayerNorm with
    # gamma folded into the next weights; leaky + fused next-layer matmul,
    # or log_softmax for the final layer.
    ob = sb.tile([P, CO], BF16, tag="ob")
    rs = sb.tile([P, 1], F32, tag="rs")
    if use_den:
        denr = sb.tile([P, Hn], F32, tag="denr")
        nc.vector.tensor_scalar(out=denr[:], in0=den[:],
                                scalar1=1e-16, scalar2=None, op0=AL.add)
        rec = sb.tile([P, Hn], F32, tag="rec")
        nc.vector.reciprocal(rec[:], denr[:])
        nc.vector.tensor_tensor(
            out=ob[:].rearrange("p (h c) -> p h c", h=Hn),
            in0=agg[:].rearrange("p (h c) -> p h c", h=Hn),
            in1=rec[:].to_broadcast([P, Hn, Ch]), op=AL.mult)
        nc.vector.tensor_reduce(out=rs[:], in_=ob[:],
                                axis=mybir.AxisListType.X, op=AL.add)
    else:
        nc.vector.tensor_scalar(out=ob[:], in0=agg[:, 0:CO], scalar1=1.0,
                                scalar2=0.0, op0=AL.mult, op1=AL.add,
                                accum_out=rs[:])
    if brow_sb is not None:
        ob2 = sb.tile([P, CO], BF16, tag="ob2")
        nc.vector.tensor_tensor(out=ob2[:], in0=ob[:], in1=brow_sb[:],
                                op=AL.add)
        ob = ob2
        rs2 = sb.tile([P, 1], F32, tag="rs2")
        nc.vector.tensor_reduce(out=rs2[:], in_=ob[:],
                                axis=mybir.AxisListType.X, op=AL.add)
        rs = rs2
    # LayerNorm statistics: mean, then var = sum((x-mu)*x)/CO
    nm = sb.tile([P, 1], F32, tag="nm")
    nc.vector.tensor_scalar(out=nm[:], in0=rs[:], scalar1=1.0 / CO,
                            scalar2=None, op0=AL.mult)
    sqs = sb.tile([P, CO], BF16, tag="sqs")
    vs = sb.tile([P, 1], F32, tag="vs")
    nc.vector.scalar_tensor_tensor(
        out=sqs[:], in0=ob[:], scalar=nm[:, 0:1], in1=ob[:],
        op0=AL.subtract, op1=AL.mult, accum_out=vs[:])
    lnv = sb.tile([P, 1], F32, tag="lnv")
    nc.scalar.activation(lnv[:], vs[:], AF.Ln, bias=LN_EPS, scale=1.0 / CO)
    rstd = sb.tile([P, 1], F32, tag="rstd")
    nc.scalar.activation(rstd[:], lnv[:], AF.Exp, bias=0.0, scale=-0.5)
    y1 = sb.tile([P, CO], BF16, tag="y1")
    nc.vector.tensor_scalar(out=y1[:], in0=ob[:], scalar1=nm[:, 0:1],
                            scalar2=rstd[:, 0:1], op0=AL.subtract,
                            op1=AL.mult)
    if grow_sb is not None:
        yg = sb.tile([P, CO], BF16, tag="yg")
        nc.vector.tensor_tensor(out=yg[:], in0=y1[:], in1=grow_sb[:],
                                op=AL.mult)
        y1 = yg

    if final:
        # y2 = y1*g3 + b3, then log_softmax over CO, write y (f32 math)
        yg3 = sb.tile([P, CO], F32, tag="yg3")
        nc.vector.tensor_tensor(out=yg3[:], in0=y1[:],
                                in1=g3b3_sb[:, 0:CO], op=AL.mult)
        yb3 = sb.tile([P, CO], F32, tag="yb3")
        nc.vector.tensor_tensor(out=yb3[:], in0=yg3[:],
                                in1=g3b3_sb[:, CO:2 * CO], op=AL.add)
        mx = sb.tile([P, 1], F32, tag="mx")
        nc.vector.tensor_reduce(out=mx[:], in_=yb3[:],
                                axis=mybir.AxisListType.X, op=AL.max)
        xs = sb.tile([P, CO], F32, tag="xs")
        nc.vector.tensor_scalar(out=xs[:], in0=yb3[:], scalar1=mx[:, 0:1],
                                scalar2=None, op0=AL.subtract)
        ex = sb.tile([P, CO], F32, tag="ex")
        se = sb.tile([P, 1], F32, tag="se")
        nc.scalar.activation(ex[:], xs[:], AF.Exp, accum_out=se[:])
        lse = sb.tile([P, 1], F32, tag="lse")
        nc.scalar.activation(lse[:], se[:], AF.Ln)
        yo = sb.tile([P, CO], F32, tag="yo")
        nc.vector.tensor_scalar(out=yo[:], in0=xs[:], scalar1=lse[:, 0:1],
                                scalar2=None, op0=AL.subtract)
        nc.sync.dma_start(y[t * P:(t + 1) * P, :], yo[:])
        return

    # leaky(0.01) -> bf16 x_next; fused next-layer matmul
    x2 = sb.tile([P, CO], BF16, tag="x2")
    nc.vector.scalar_tensor_tensor(
        out=x2[:], in0=y1[:], scalar=NEG_SLOPE_ACT, in1=y1[:],
        op0=AL.mult, op1=AL.max)
    W_sb, KC, Ww = fuse["W_sb"], fuse["KC"], fuse["Ww"]
    CO2, Hn2, ELEMn = fuse["CO"], fuse["Hn2"], fuse["ELEMn"]
    xt2 = sb.tile([P, KC * P], BF16, tag="xt2")
    for k in range(KC):
        scr = psT.tile([P, P], BF16, tag="scr")
        nc.tensor.transpose(out=scr[:], in_=x2[:, k * P:(k + 1) * P],
                            identity=id_sb[:])
        nc.scalar.copy(xt2[:, k * P:(k + 1) * P], scr[:])
    hp = ps1.tile([P, Ww], F32, tag="hnext")
    _mm_splits(nc, hp, xt2, W_sb, KC, Ww, P)
    _store_hs(nc, sb, hp, CO2, Hn2, ELEMn, fuse["hs_loc"], fuse["d_next"], t)


# --------------------------------------------------------------------------
# entry point
# --------------------------------------------------------------------------

_CACHE = {}


def _get_nc(cfg, meta):
    key = (tuple(sorted(cfg.__dict__.items())),
           tuple(tuple(r) for r in meta.nch), meta.bias12, meta.gfold)
    if key not in _CACHE:
        _CACHE[key] = build_nc(cfg, meta)
    return _CACHE[key]


def kernel(**inputs):
    inputs = {k: np.asarray(v) for k, v in inputs.items()}
    x = inputs["x"]
    cfg = Cfg(N=x.shape[0], E=inputs["edge_src"].shape[0], F_IN=x.shape[1],
              HEADS=inputs["a_src1"].shape[0], C1=inputs["a_src1"].shape[1],
              C2=inputs["a_src2"].shape[1], NCLS=inputs["W3"].shape[0],
              NCORES=8)
    in_maps, meta = host_prep(cfg, **inputs)
    nc = _get_nc(cfg, meta)
    trace = bool(int(os.environ.get("GAT_TRACE", "0")))
    res = run_bass_kernel_spmd(nc, in_maps, core_ids=list(range(cfg.NCORES)),
                               trace=trace)
    global LAST_EXEC_NS
    LAST_EXEC_NS = res.exec_time_ns
    out = np.concatenate(
        [res.results[cc]["y"][:cfg.NL] for cc in range(cfg.NCORES)], axis=0)
    return out.astype(np.float32)


LAST_EXEC_NS = None


if __name__ == "__main__":
    pass
